# revision 5
# baseline (speedup 1.0000x reference)
"""Trainium2 Bass kernel for nn_ChannelAdaptiveNormalization.

Reference computation (per batch):
    src_n = instnorm(src); q = Wq@src_n; k = Wk@instnorm(trg); v = Wv@trg
    attn = softmax(q^T k / sqrt(C))  over t
    mean = attn @ v ; var = relu(attn @ v^2 - mean^2)
    out = sqrt(mean_s[var]) * src_n + mean_s[mean]      (broadcast over time)

Kernel decomposition (all per-core, data-parallel over batch, 2 batches/core):
  * instance-norm is folded into the CxC projection weights (scale columns by
    1/sd, subtract a rank-1 bias) -- normalized activations never materialize.
  * scores are produced TRANSPOSED ([t, s]) so the attn contraction over t
    needs no transposes; softmax uses exp without max subtraction (scores are
    ~N(0,1)); Z (softmax denominators) via a replicated ones-matmul.
  * only column-reductions of mean/var over s are needed, so the full
    mean matrix is reduced on the fly from PSUM; attn@v^2 collapses to
    a tiny matvec with a_u[t] = sum_s attn[t,s].
  * final output is a per-(b,c) affine of raw src: out = A*src + B.

Scheduling (v2): the PE HAM clock-gate runs the array at 1.2 GHz until it
sees ~3.4us of sustained matmul work, and re-throttles after any ~3.4us idle
gap.  The emission order therefore software-pipelines the two batches and
the two s-halves so the PE queue never stalls more than ~1us: scores of the
next half are emitted before the softmax-normalize/AV of the previous one,
and batch 1's projections fill the PE gap while batch 0's AV waits on the
DVE normalize.  Dummy matmuls paced through the (DMA-bound) prologue keep
the HAM warm before the first projection burst.
"""

import os
import sys

import numpy as np

if "/opt/trn_rl_repo" not in sys.path:
    sys.path.insert(0, "/opt/trn_rl_repo")

from contextlib import ExitStack

import concourse.bass as bass
import concourse.tile as tile
from concourse import mybir
from concourse.bass_utils import run_bass_kernel_spmd

DT = mybir.dt
ALU = mybir.AluOpType
ACTF = mybir.ActivationFunctionType

N_CORES = 8
B_FULL = 16
B_SH = B_FULL // N_CORES  # 2 batches per core
C = 256
T = 2048
P = 128
NCH = C // P  # 2 channel chunks
NTCH = T // P  # 16 time chunks
EPS = 1e-5


def _build_nc() -> bass.Bass:
    nc = bass.Bass()
    src = nc.declare_dram_parameter("src", [B_SH, C, T], DT.float32, isOutput=False)
    trg = nc.declare_dram_parameter("trg", [B_SH, C, T], DT.float32, isOutput=False)
    wqt = nc.declare_dram_parameter("wqt", [C, C], DT.float32, isOutput=False)
    wkt = nc.declare_dram_parameter("wkt", [C, C], DT.float32, isOutput=False)
    wvt = nc.declare_dram_parameter("wvt", [C, C], DT.float32, isOutput=False)
    out = nc.declare_dram_parameter("out", [B_SH, C, T], DT.float32, isOutput=True)

    with tile.TileContext(nc) as tc:
        with ExitStack() as ctx:
            _build_kernel(ctx, tc, src, trg, wqt, wkt, wvt, out)
    _legalize_waits(nc)
    return nc


def _legalize_waits(nc: bass.Bass):
    """walrus on this toolchain encodes at most ONE sync wait per
    instruction (NEURON_ISA_TPB_EVENTS has a single wait slot and no
    splitting pass runs).  Hoist all but the last wait of every
    instruction into standalone single-wait EventSemaphore instructions
    on the same engine queue, which preserves ordering semantics."""
    # collect all tile-context data semaphores (skip barrier sems)
    all_sems = {}
    for fn in nc.m.functions:
        for blk in fn.blocks:
            for inst in blk.instructions:
                si = getattr(inst, "sync_info", None)
                if si is None:
                    continue
                for w in list(si.on_wait) + list(si.on_update):
                    if not w.ant_name.startswith("barrier"):
                        all_sems[w.id] = w.ant_name

    for fn in nc.m.functions:
        for blk in fn.blocks:
            snapshot = list(blk.instructions)
            for idx in range(len(snapshot) - 1, -1, -1):
                inst = snapshot[idx]
                if type(inst).__name__ == "InstISA" and getattr(inst, "isa_opcode", None) == 176:
                    # EVENT_SEMAPHORE_RANGE_CLEAR: encoding mismatches this
                    # walrus build; replace with per-sem zero-writes.
                    pos = list(blk.instructions).index(inst)
                    blk.instructions.pop(pos)
                    for sid, sname in sorted(all_sems.items()):
                        ev = mybir.InstEventSemaphore(
                            name=nc.get_next_instruction_name(), ins=[], outs=[]
                        )
                        ev.engine = inst.engine
                        ev.sync_info = mybir.SyncInfo(
                            on_wait=[],
                            on_update=[
                                mybir.SyncUpdate(
                                    sync_type="semaphore",
                                    id=sid,
                                    ant_name=sname,
                                    update_mode="sem-wr-imm",
                                    update_value=0,
                                )
                            ],
                        )
                        nc.register_instruction(ev)
                        blk.instructions.insert(pos, ev)
                        pos += 1

    for fn in nc.m.functions:
        for blk in fn.blocks:
            snapshot = list(blk.instructions)
            for idx in range(len(snapshot) - 1, -1, -1):
                inst = snapshot[idx]
                si = getattr(inst, "sync_info", None)
                if si is None or len(si.on_wait) <= 1:
                    continue
                waits = list(si.on_wait)
                evs = []
                for w in waits[:-1]:
                    ev = mybir.InstEventSemaphore(
                        name=nc.get_next_instruction_name(), ins=[], outs=[]
                    )
                    ev.engine = inst.engine
                    ev.sync_info = mybir.SyncInfo(on_wait=[w], on_update=[])
                    nc.register_instruction(ev)
                    evs.append(ev)
                si.on_wait = waits[-1:]
                inst.sync_info = si
                for ev in reversed(evs):
                    blk.instructions.insert(idx, ev)


def _build_kernel(ctx, tc, src, trg, wqt, wkt, wvt, out):
    nc = tc.nc
    ep = ctx.enter_context

    pool_const = ep(tc.tile_pool(name="const", bufs=1))
    pool_wtmp = ep(tc.tile_pool(name="wtmp", bufs=1))
    pool_sf = ep(tc.tile_pool(name="sf", bufs=1))
    pool_tf = ep(tc.tile_pool(name="tf", bufs=1))
    pool_sbf = ep(tc.tile_pool(name="sbf", bufs=2))
    pool_tbf = ep(tc.tile_pool(name="tbf", bufs=1))
    pool_qk = ep(tc.tile_pool(name="qk", bufs=1))
    pool_v = ep(tc.tile_pool(name="vpool", bufs=1))
    # 32 bufs: both s-halves' p tiles must be live simultaneously, since
    # half 1's scores are emitted before half 0's AV (a smaller ring makes
    # half-1 exp wait on half-0 AV matmuls that sit *behind* half-1's Z
    # matmuls in the in-order PE queue -> deadlock).
    pool_p = ep(tc.tile_pool(name="ppool", bufs=32))
    pool_zi = ep(tc.tile_pool(name="zipool", bufs=2))
    pool_stat = ep(tc.tile_pool(name="stat", bufs=2))
    pool_stat2 = ep(tc.tile_pool(name="stat2", bufs=2))
    pool_junk = ep(tc.tile_pool(name="junk", bufs=2))
    pool_out = ep(tc.tile_pool(name="outio", bufs=2))
    ps_s = ep(tc.tile_pool(name="ps_s", bufs=2, space="PSUM"))
    ps_z = ep(tc.tile_pool(name="ps_z", bufs=2, space="PSUM"))

    # ---- constants / weights (once) ----
    ones_bf = pool_const.tile([P, P], DT.bfloat16, name="ones_bf")
    nc.vector.memset(ones_bf[:], 1.0)
    dummy_bf = pool_const.tile([P, 512], DT.bfloat16, name="dummy_bf")
    nc.vector.memset(dummy_bf[:], 0.0)

    def warm_mm(rhs_ap=None):
        """One dummy matmul to keep the PE HAM clock-gate warm.  If rhs_ap
        is given the matmul waits for it, pacing the dummy to just after
        that producer; otherwise it runs as soon as the PE is free."""
        dps = ps_s.tile([P, 512], DT.float32, name="sps", tag="sps")
        nc.tensor.matmul(
            dps[:],
            lhsT=ones_bf[:],
            rhs=rhs_ap if rhs_ap is not None else dummy_bf[:],
            start=True,
            stop=True,
            skip_group_check=True,
        )

    # weight layout in SBUF: [128 (c within chunk), NCH*C (cchunk-major, d)]
    wq_bf = pool_const.tile([P, NCH * C], DT.bfloat16, name="wq_bf")
    wk_bf = pool_const.tile([P, NCH * C], DT.bfloat16, name="wk_bf")
    wv_bf = pool_const.tile([P, NCH * C], DT.bfloat16, name="wv_bf")
    for w_bf, w_d, wnm in ((wq_bf, wqt, "q"), (wk_bf, wkt, "k"), (wv_bf, wvt, "v")):
        wtmp = pool_wtmp.tile([P, NCH * C], DT.float32, name=f"wtmp_{wnm}")
        nc.gpsimd.dma_start(
            wtmp[:].rearrange("p (a d) -> p a d", a=NCH),
            w_d[:].rearrange("(a p) d -> p a d", p=P),
        )
        nc.vector.tensor_copy(w_bf[:], wtmp[:])

    # ~4.3us of back-to-back dummy matmuls at kernel start: push the HAM
    # through its cold window while the first DMAs are in flight.
    for _ in range(10):
        warm_mm()

    st = [dict() for _ in range(B_SH)]

    def prologue_io(b, fast):
        """DMA loads + f32->bf16 casts.  fast=True (batch 0, nothing else
        running) splits casts across DVE and ScalarE; otherwise they go to
        the idle GpSimd engine so DVE/ScalarE stay free for compute."""
        s = st[b]
        s["s_f"], s["t_f"], s["s_bf"], s["t_bf"] = [], [], [], []
        for cc in range(NCH):
            sf = pool_sf.tile([P, T], DT.float32, name=f"s_f{cc}")
            nc.gpsimd.dma_start(sf[:], src[b, cc * P : (cc + 1) * P, :])
            s["s_f"].append(sf)
            tf = pool_tf.tile([P, T], DT.float32, name=f"t_f{cc}")
            nc.sync.dma_start(tf[:], trg[b, cc * P : (cc + 1) * P, :])
            s["t_f"].append(tf)
        for cc in range(NCH):
            sb = pool_sbf.tile([P, T], DT.bfloat16, name=f"s_bf{cc}")
            tb = pool_tbf.tile([P, T], DT.bfloat16, name=f"t_bf{cc}")
            if fast:
                nc.vector.tensor_copy(sb[:], s["s_f"][cc][:])
                warm_mm(sb[:, 0:512])
                nc.scalar.copy(tb[:], s["t_f"][cc][:])
                warm_mm(tb[:, 0:512])
            else:
                nc.gpsimd.tensor_copy(sb[:], s["s_f"][cc][:])
                nc.gpsimd.tensor_copy(tb[:], s["t_f"][cc][:])
            s["s_bf"].append(sb)
            s["t_bf"].append(tb)

    def rowstats(b, x_bf, nm):
        """-> (mean [P,1] f32 AP, inv_sd [P,1] f32 tile) per row over T."""
        bnst = pool_stat.tile([P, 4 * 6], DT.float32, name=f"bnst_{nm}")
        for j in range(4):
            nc.vector.bn_stats(bnst[:, 6 * j : 6 * (j + 1)], x_bf[:, 512 * j : 512 * (j + 1)])
        mv = pool_stat.tile([P, 2], DT.float32, name=f"mv_{nm}")
        nc.vector.bn_aggr(mv[:], bnst[:])
        sd = pool_stat.tile([P, 1], DT.float32, name=f"sd_{nm}")
        # sd = sqrt(var_pop * T/(T-1)) + EPS
        nc.scalar.activation(sd[:], mv[:, 1:2], ACTF.Sqrt, scale=float(T) / (T - 1))
        sde = pool_stat.tile([P, 1], DT.float32, name=f"sde_{nm}")
        nc.vector.tensor_scalar_add(sde[:], sd[:], EPS)
        inv = pool_stat.tile([P, 1], DT.float32, name=f"inv_{nm}")
        nc.vector.reciprocal(inv[:], sde[:])
        return mv[:, 0:1], inv

    def prologue_stats(b):
        s = st[b]
        s["mean_s"], s["inv_s"], s["mean_t"], s["inv_t"] = [], [], [], []
        for cc in range(NCH):
            m, i = rowstats(b, s["s_bf"][cc], f"s{cc}")
            s["mean_s"].append(m)
            s["inv_s"].append(i)
            m, i = rowstats(b, s["t_bf"][cc], f"t{cc}")
            s["mean_t"].append(m)
            s["inv_t"].append(i)

        # wq_s[c, d] = wqt[c, d] * inv_s[c]  (bf16), same for wk_s with inv_t
        wq_s = pool_stat.tile([P, NCH * C], DT.bfloat16, name="wq_s")
        wk_s = pool_stat.tile([P, NCH * C], DT.bfloat16, name="wk_s")
        mi_s, mi_t = [], []
        for cc in range(NCH):
            nc.vector.tensor_scalar_mul(
                wq_s[:, cc * C : (cc + 1) * C], wq_bf[:, cc * C : (cc + 1) * C], s["inv_s"][cc][:]
            )
            nc.vector.tensor_scalar_mul(
                wk_s[:, cc * C : (cc + 1) * C], wk_bf[:, cc * C : (cc + 1) * C], s["inv_t"][cc][:]
            )
            mis = pool_stat2.tile([P, 1], DT.bfloat16, name=f"mi_s{cc}")
            nc.vector.tensor_tensor(mis[:], s["mean_s"][cc], s["inv_s"][cc][:], ALU.mult)
            mi_s.append(mis)
            mit = pool_stat2.tile([P, 1], DT.bfloat16, name=f"mi_t{cc}")
            nc.vector.tensor_tensor(mit[:], s["mean_t"][cc], s["inv_t"][cc][:], ALU.mult)
            mi_t.append(mit)
        s["wq_s"], s["wk_s"], s["mi_s"], s["mi_t"] = wq_s, wk_s, mi_s, mi_t

        # PE pre-touches: pull cross-engine operand-ready waits off the first
        # real matmuls (MM encoding allows at most 2 sync waits).
        for ap in (s["s_bf"][0], s["s_bf"][1], s["t_bf"][0], s["t_bf"][1]):
            nc.tensor.ldweights(weights=ap[:, 0:P])
        for ap in (wq_s, wk_s):
            nc.tensor.ldweights(weights=ap[:, 0:P])
        for ap in (mi_s[0], mi_s[1], mi_t[0], mi_t[1]):
            nc.tensor.ldweights(weights=ap[:])

        # beta[d] = sum_c w_s[c,d] * (mu[c]*inv[c]); psum [P, NCH] (d-chunk cols)
        negb = []
        for w_s, mi, nm in ((wq_s, mi_s, "q"), (wk_s, mi_t, "k")):
            bps = ps_s.tile([P, NCH], DT.float32, name="sps", tag="sps")
            for dc in range(NCH):
                for cc in range(NCH):
                    nc.tensor.matmul(
                        bps[:, dc : dc + 1],
                        lhsT=w_s[:, cc * C + dc * P : cc * C + (dc + 1) * P],
                        rhs=mi[cc][:],
                        start=(cc == 0),
                        stop=(cc == NCH - 1),
                    )
            nb = pool_stat2.tile([P, NCH], DT.float32, name=f"negb_{nm}")
            nc.vector.tensor_scalar_mul(nb[:], bps[:], -1.0)
            negb.append(nb)
        s["negbq"], s["negbk"] = negb

    def proj_qk(b):
        # Qt/Kt: [d, t] bf16 (per d-chunk tiles), bias folded during eviction
        s = st[b]
        s["qt_bf"], s["kt_bf"] = [], []
        for w_s, nb, outk, nm in (
            (s["wq_s"], s["negbq"], "qt_bf", "qt"),
            (s["wk_s"], s["negbk"], "kt_bf", "kt"),
        ):
            x_bf = s["s_bf"] if nm == "qt" else s["t_bf"]
            for dc in range(NCH):
                ot = pool_qk.tile([P, T], DT.bfloat16, name=f"{nm}{dc}")
                for half in range(2):
                    pps = ps_s.tile([P, 1024], DT.float32, name="sps", tag="sps")
                    for cc in range(NCH):
                        for n4 in range(2):
                            nc.tensor.matmul(
                                pps[:, 512 * n4 : 512 * (n4 + 1)],
                                lhsT=w_s[:, cc * C + dc * P : cc * C + (dc + 1) * P],
                                rhs=x_bf[cc][:, 1024 * half + 512 * n4 : 1024 * half + 512 * (n4 + 1)],
                                start=(cc == 0),
                                stop=(cc == NCH - 1),
                            )
                    nc.scalar.activation(
                        ot[:, 1024 * half : 1024 * (half + 1)],
                        pps[:],
                        ACTF.Identity,
                        bias=nb[:, dc : dc + 1],
                        scale=1.0,
                    )
                s[outk].append(ot)

    def proj_v(b):
        # V_T: [t within chunk, tchunk-major d]  (v_bf[p, 256*j + d])
        s = st[b]
        v_bf = pool_v.tile([P, NTCH * C], DT.bfloat16, name="v_bf")
        v2_bf = pool_v.tile([P, NTCH * C], DT.bfloat16, name="v2_bf")
        for g in range(4):
            vps = ps_s.tile([P, 1024], DT.float32, name="sps", tag="sps")
            for j4 in range(4):
                j = 4 * g + j4
                for cc in range(NCH):
                    nc.tensor.matmul(
                        vps[:, 256 * j4 : 256 * (j4 + 1)],
                        lhsT=s["t_bf"][cc][:, P * j : P * (j + 1)],
                        rhs=wv_bf[:, cc * C : (cc + 1) * C],
                        start=(cc == 0),
                        stop=(cc == NCH - 1),
                    )
            nc.scalar.copy(v_bf[:, 1024 * g : 1024 * (g + 1)], vps[:])
        nc.vector.tensor_mul(v2_bf[:], v_bf[:], v_bf[:])
        s["v_bf"], s["v2_bf"] = v_bf, v2_bf

    def stats_init(b):
        s = st[b]
        s["sm_h"] = pool_stat.tile([P, 2 * NCH], DT.float32, name="sm_h")
        s["sm2_h"] = pool_stat.tile([P, 2 * NCH], DT.float32, name="sm2_h")
        s["a_uh"] = pool_stat.tile([P, 2 * NTCH], DT.float32, name="a_uh")
        s["p_t"] = [[], []]
        s["z_ps"] = [None, None]

    def scores_half(b, sh):
        """scores^T -> exp -> Z accumulation for s-half sh."""
        s = st[b]
        so = 1024 * sh
        z_ps = ps_z.tile([P, 1024], DT.float32, name="zav", tag="zav")
        s["z_ps"][sh] = z_ps
        for tch in range(NTCH):
            p = pool_p.tile([P, 1024], DT.bfloat16, name="p")
            sps = ps_s.tile([P, 1024], DT.float32, name="sps", tag="sps")
            for dc in range(NCH):
                for n2 in range(2):
                    nc.tensor.matmul(
                        sps[:, 512 * n2 : 512 * (n2 + 1)],
                        lhsT=s["kt_bf"][dc][:, P * tch : P * (tch + 1)],
                        rhs=s["qt_bf"][dc][:, so + 512 * n2 : so + 512 * (n2 + 1)],
                        start=(dc == 0),
                        stop=(dc == NCH - 1),
                    )
            nc.scalar.activation(p[:], sps[:], ACTF.Exp, scale=1.0 / 16.0)
            for n2 in range(2):
                nc.tensor.matmul(
                    z_ps[:, 512 * n2 : 512 * (n2 + 1)],
                    lhsT=ones_bf[:],
                    rhs=p[:, 512 * n2 : 512 * (n2 + 1)],
                    start=(tch == 0),
                    stop=(tch == NTCH - 1),
                    skip_group_check=True,
                )
            s["p_t"][sh].append(p)

    def av_half(b, sh):
        """normalize p by 1/Z (accumulating a_u) and contract attn @ v."""
        s = st[b]
        z_ps = s["z_ps"][sh]
        # NB: reciprocal_approx_fast (custom DVE op) fails neuronxcc codegen
        # on this toolchain ("ISA wrong length"), so use the stock op.
        zinv = pool_zi.tile([P, 1024], DT.float32, name="zinv")
        nc.vector.reciprocal(zinv[:], z_ps[:])

        for i, p in enumerate(s["p_t"][sh]):
            # out = (P * 1) * zinv (normalize in place); accum -> a_u half
            nc.vector.scalar_tensor_tensor(
                out=p[:],
                in0=p[:],
                scalar=1.0,
                in1=zinv[:],
                op0=ALU.mult,
                op1=ALU.mult,
                accum_out=s["a_uh"][:, NTCH * sh + i : NTCH * sh + i + 1],
            )

        for dc in range(NCH):
            avps = ps_s.tile([P, 1024], DT.float32, name="sps", tag="sps")
            for tch in range(NTCH):
                for n2 in range(2):
                    nc.tensor.matmul(
                        avps[:, 512 * n2 : 512 * (n2 + 1)],
                        lhsT=s["v_bf"][:, 256 * tch + P * dc : 256 * tch + P * (dc + 1)],
                        rhs=s["p_t"][sh][tch][:, 512 * n2 : 512 * (n2 + 1)],
                        start=(tch == 0),
                        stop=(tch == NTCH - 1),
                    )
            junk = pool_junk.tile([P, 1024], DT.bfloat16, name="junk")
            nc.scalar.activation(
                junk[:], avps[:], ACTF.Square,
                accum_out=s["sm2_h"][:, NCH * sh + dc : NCH * sh + dc + 1],
            )
            nc.vector.reduce_sum(
                s["sm_h"][:, NCH * sh + dc : NCH * sh + dc + 1], avps[:],
                axis=mybir.AxisListType.X,
            )

    def finals(b):
        s = st[b]
        # combine halves
        a_u = pool_stat.tile([P, NTCH], DT.float32, name="a_u")
        nc.vector.tensor_add(a_u[:], s["a_uh"][:, 0:NTCH], s["a_uh"][:, NTCH : 2 * NTCH])
        a_ub = pool_stat2.tile([P, NTCH], DT.bfloat16, name="a_ub")
        nc.vector.tensor_copy(a_ub[:], a_u[:])
        sm = pool_stat.tile([P, NCH], DT.float32, name="sm")
        nc.vector.tensor_add(sm[:], s["sm_h"][:, 0:NCH], s["sm_h"][:, NCH : 2 * NCH])
        sm2 = pool_stat.tile([P, NCH], DT.float32, name="sm2")
        nc.vector.tensor_add(sm2[:], s["sm2_h"][:, 0:NCH], s["sm2_h"][:, NCH : 2 * NCH])

        # attn@v^2 matvec (per-partition result)
        av2_ps = ps_s.tile([P, NCH], DT.float32, name="sps", tag="sps")
        for dc in range(NCH):
            for j in range(NTCH):
                nc.tensor.matmul(
                    av2_ps[:, dc : dc + 1],
                    lhsT=s["v2_bf"][:, 256 * j + P * dc : 256 * j + P * (dc + 1)],
                    rhs=a_ub[:, j : j + 1],
                    start=(j == 0),
                    stop=(j == NTCH - 1),
                )

        # ================= finals + output ================================
        for dc in range(NCH):
            d1 = pool_stat.tile([P, 1], DT.float32, name=f"d1_{dc}")
            nc.vector.tensor_tensor(d1[:], av2_ps[:, dc : dc + 1], sm2[:, dc : dc + 1], ALU.subtract)
            r1 = pool_stat.tile([P, 1], DT.float32, name=f"r1_{dc}")
            nc.vector.tensor_scalar_max(r1[:], d1[:], 0.0)
            stdv = pool_stat.tile([P, 1], DT.float32, name=f"std_{dc}")
            nc.scalar.activation(stdv[:], r1[:], ACTF.Sqrt, scale=1.0 / T)
            av = pool_stat.tile([P, 1], DT.float32, name=f"av_{dc}")
            nc.vector.tensor_tensor(av[:], stdv[:], s["inv_s"][dc][:], ALU.mult)
            musc = pool_stat.tile([P, 1], DT.float32, name=f"musc_{dc}")
            nc.vector.tensor_scalar_mul(musc[:], sm[:, dc : dc + 1], 1.0 / T)
            negms = pool_stat.tile([P, 1], DT.float32, name=f"negms_{dc}")
            nc.vector.tensor_scalar_mul(negms[:], s["mean_s"][dc], -1.0)
            bv = pool_stat.tile([P, 1], DT.float32, name=f"bv_{dc}")
            nc.vector.scalar_tensor_tensor(
                out=bv[:], in0=av[:], scalar=negms[:], in1=musc[:], op0=ALU.mult, op1=ALU.add
            )
            for half in range(2):
                o_sb = pool_out.tile([P, 1024], DT.float32, name="o_sb")
                nc.scalar.activation(
                    o_sb[:],
                    s["s_bf"][dc][:, 1024 * half : 1024 * (half + 1)],
                    ACTF.Identity,
                    bias=bv[:],
                    scale=av[:],
                )
                nc.sync.dma_start(
                    out[b, dc * P : (dc + 1) * P, 1024 * half : 1024 * (half + 1)], o_sb[:]
                )

    # ================= software-pipelined emission ======================
    prologue_io(0, fast=True)
    prologue_stats(0)
    stats_init(0)
    proj_qk(0)
    proj_v(0)
    scores_half(0, 0)
    prologue_io(1, fast=False)
    scores_half(0, 1)
    av_half(0, 0)
    prologue_stats(1)
    stats_init(1)
    proj_qk(1)
    av_half(0, 1)
    finals(0)
    # proj_v(1) must come after finals(0): v2_bf(1)'s DVE write reuses
    # v2_bf(0)'s slot, whose last reader is the av2 matvec in finals(0) --
    # and that matvec needs a_ub from DVE instructions that would otherwise
    # queue behind the v2 multiply.
    proj_v(1)
    scores_half(1, 0)
    scores_half(1, 1)
    av_half(1, 0)
    av_half(1, 1)
    finals(1)


_NC_CACHE = None


def _get_nc():
    global _NC_CACHE
    if _NC_CACHE is None:
        _NC_CACHE = _build_nc()
    return _NC_CACHE


def _run(src, trg, Wq, Wk, Wv, **kwargs):
    src = np.ascontiguousarray(np.asarray(src, dtype=np.float32))
    trg = np.ascontiguousarray(np.asarray(trg, dtype=np.float32))
    wqt = np.ascontiguousarray(np.asarray(Wq, dtype=np.float32).T)
    wkt = np.ascontiguousarray(np.asarray(Wk, dtype=np.float32).T)
    wvt = np.ascontiguousarray(np.asarray(Wv, dtype=np.float32).T)
    nc = _get_nc()
    in_maps = [
        {
            "src": src[i * B_SH : (i + 1) * B_SH],
            "trg": trg[i * B_SH : (i + 1) * B_SH],
            "wqt": wqt,
            "wkt": wkt,
            "wvt": wvt,
        }
        for i in range(N_CORES)
    ]
    res = run_bass_kernel_spmd(nc, in_maps, list(range(N_CORES)), **kwargs)
    outp = np.concatenate([res.results[i]["out"] for i in range(N_CORES)], axis=0)
    return outp.astype(np.float32), res


def kernel(src, trg, Wq, Wk, Wv):
    outp, _ = _run(src, trg, Wq, Wk, Wv)
    return outp


# revision 14
# speedup vs baseline: 1.0511x; 1.0511x over previous
"""Trainium2 Bass kernel for nn_ChannelAdaptiveNormalization.

Reference computation (per batch):
    src_n = instnorm(src); q = Wq@src_n; k = Wk@instnorm(trg); v = Wv@trg
    attn = softmax(q^T k / sqrt(C))  over t
    mean = attn @ v ; var = relu(attn @ v^2 - mean^2)
    out = sqrt(mean_s[var]) * src_n + mean_s[mean]      (broadcast over time)

Kernel decomposition (all per-core, data-parallel over batch, 2 batches/core):
  * instance-norm is folded into the CxC projection weights (scale columns by
    1/sd, subtract a rank-1 bias) -- normalized activations never materialize.
  * scores are produced TRANSPOSED ([t, s]) so the attn contraction over t
    needs no transposes; softmax uses exp without max subtraction (scores are
    ~N(0,1)); Z (softmax denominators) via a replicated ones-matmul.
  * only column-reductions of mean/var over s are needed, so the full
    mean matrix is reduced on the fly from PSUM; attn@v^2 collapses to
    a tiny matvec with a_u[t] = sum_s attn[t,s].
  * final output is a per-(b,c) affine of raw src: out = A*src + B.

Scheduling (v2): the PE HAM clock-gate runs the array at 1.2 GHz until it
sees ~3.4us of sustained matmul work, and re-throttles after any ~3.4us idle
gap.  The emission order therefore software-pipelines the two batches and
the two s-halves so the PE queue never stalls more than ~1us: scores of the
next half are emitted before the softmax-normalize/AV of the previous one,
and batch 1's projections fill the PE gap while batch 0's AV waits on the
DVE normalize.  Dummy matmuls paced through the (DMA-bound) prologue keep
the HAM warm before the first projection burst.
"""

import os
import sys

import numpy as np

if "/opt/trn_rl_repo" not in sys.path:
    sys.path.insert(0, "/opt/trn_rl_repo")

from contextlib import ExitStack

import concourse.bass as bass
import concourse.tile as tile
from concourse import mybir
from concourse.bass_utils import run_bass_kernel_spmd

DT = mybir.dt
ALU = mybir.AluOpType
ACTF = mybir.ActivationFunctionType

N_CORES = 8
B_FULL = 16
B_SH = B_FULL // N_CORES  # 2 batches per core
C = 256
T = 2048
P = 128
NCH = C // P  # 2 channel chunks
NTCH = T // P  # 16 time chunks
EPS = 1e-5


def _build_nc() -> bass.Bass:
    nc = bass.Bass()
    src = nc.declare_dram_parameter("src", [B_SH, C, T], DT.float32, isOutput=False)
    trg = nc.declare_dram_parameter("trg", [B_SH, C, T], DT.float32, isOutput=False)
    wqt = nc.declare_dram_parameter("wqt", [C, C], DT.float32, isOutput=False)
    wkt = nc.declare_dram_parameter("wkt", [C, C], DT.float32, isOutput=False)
    wvt = nc.declare_dram_parameter("wvt", [C, C], DT.float32, isOutput=False)
    out = nc.declare_dram_parameter("out", [B_SH, C, T], DT.float32, isOutput=True)

    with tile.TileContext(nc) as tc:
        with ExitStack() as ctx:
            _build_kernel(ctx, tc, src, trg, wqt, wkt, wvt, out)
    _legalize_waits(nc)
    return nc


def _legalize_waits(nc: bass.Bass):
    """walrus on this toolchain encodes at most ONE sync wait per
    instruction (NEURON_ISA_TPB_EVENTS has a single wait slot and no
    splitting pass runs).  Hoist all but the last wait of every
    instruction into standalone single-wait EventSemaphore instructions
    on the same engine queue, which preserves ordering semantics."""
    # collect all tile-context data semaphores (skip barrier sems)
    all_sems = {}
    for fn in nc.m.functions:
        for blk in fn.blocks:
            for inst in blk.instructions:
                si = getattr(inst, "sync_info", None)
                if si is None:
                    continue
                for w in list(si.on_wait) + list(si.on_update):
                    if not w.ant_name.startswith("barrier"):
                        all_sems[w.id] = w.ant_name

    for fn in nc.m.functions:
        for blk in fn.blocks:
            snapshot = list(blk.instructions)
            for idx in range(len(snapshot) - 1, -1, -1):
                inst = snapshot[idx]
                if type(inst).__name__ == "InstISA" and getattr(inst, "isa_opcode", None) == 176:
                    # EVENT_SEMAPHORE_RANGE_CLEAR: encoding mismatches this
                    # walrus build; replace with per-sem zero-writes.
                    pos = list(blk.instructions).index(inst)
                    blk.instructions.pop(pos)
                    for sid, sname in sorted(all_sems.items()):
                        ev = mybir.InstEventSemaphore(
                            name=nc.get_next_instruction_name(), ins=[], outs=[]
                        )
                        ev.engine = inst.engine
                        ev.sync_info = mybir.SyncInfo(
                            on_wait=[],
                            on_update=[
                                mybir.SyncUpdate(
                                    sync_type="semaphore",
                                    id=sid,
                                    ant_name=sname,
                                    update_mode="sem-wr-imm",
                                    update_value=0,
                                )
                            ],
                        )
                        nc.register_instruction(ev)
                        blk.instructions.insert(pos, ev)
                        pos += 1

    for fn in nc.m.functions:
        for blk in fn.blocks:
            snapshot = list(blk.instructions)
            for idx in range(len(snapshot) - 1, -1, -1):
                inst = snapshot[idx]
                si = getattr(inst, "sync_info", None)
                if si is None or len(si.on_wait) <= 1:
                    continue
                waits = list(si.on_wait)
                evs = []
                for w in waits[:-1]:
                    ev = mybir.InstEventSemaphore(
                        name=nc.get_next_instruction_name(), ins=[], outs=[]
                    )
                    ev.engine = inst.engine
                    ev.sync_info = mybir.SyncInfo(on_wait=[w], on_update=[])
                    nc.register_instruction(ev)
                    evs.append(ev)
                si.on_wait = waits[-1:]
                inst.sync_info = si
                for ev in reversed(evs):
                    blk.instructions.insert(idx, ev)


def _build_kernel(ctx, tc, src, trg, wqt, wkt, wvt, out):
    nc = tc.nc
    ep = ctx.enter_context

    pool_const = ep(tc.tile_pool(name="const", bufs=1))
    pool_wtmp = ep(tc.tile_pool(name="wtmp", bufs=1))
    pool_sf = ep(tc.tile_pool(name="sf", bufs=1))
    pool_tf = ep(tc.tile_pool(name="tf", bufs=1))
    pool_sbf = ep(tc.tile_pool(name="sbf", bufs=2))
    pool_tbf = ep(tc.tile_pool(name="tbf", bufs=2))
    pool_qk = ep(tc.tile_pool(name="qk", bufs=1))
    pool_v = ep(tc.tile_pool(name="vpool", bufs=1))
    # 32 bufs: both s-halves' p tiles must be live simultaneously, since
    # half 1's scores are emitted before half 0's AV (a smaller ring makes
    # half-1 exp wait on half-0 AV matmuls that sit *behind* half-1's Z
    # matmuls in the in-order PE queue -> deadlock).
    pool_p = ep(tc.tile_pool(name="ppool", bufs=32))
    pool_zi = ep(tc.tile_pool(name="zipool", bufs=2))
    pool_stat = ep(tc.tile_pool(name="stat", bufs=2))
    pool_stat2 = ep(tc.tile_pool(name="stat2", bufs=2))
    pool_junk = ep(tc.tile_pool(name="junk", bufs=2))
    pool_out = ep(tc.tile_pool(name="outio", bufs=2))
    ps_s = ep(tc.tile_pool(name="ps_s", bufs=2, space="PSUM"))
    ps_z = ep(tc.tile_pool(name="ps_z", bufs=2, space="PSUM"))

    # ---- constants / weights (once) ----
    ones_bf = pool_const.tile([P, P], DT.bfloat16, name="ones_bf")
    nc.vector.memset(ones_bf[:], 1.0)
    dummy_bf = pool_const.tile([P, 512], DT.bfloat16, name="dummy_bf")
    nc.vector.memset(dummy_bf[:], 0.0)
    ones_f32 = pool_const.tile([P, P], DT.float32, name="ones_f32")
    nc.vector.memset(ones_f32[:], 1.0)

    def warm_mm(rhs_ap=None):
        """One dummy matmul to keep the PE HAM clock-gate warm.  If rhs_ap
        is given the matmul waits for it, pacing the dummy to just after
        that producer; otherwise it runs as soon as the PE is free."""
        rhs = rhs_ap if rhs_ap is not None else dummy_bf[:]
        free = rhs.shape[-1]
        lhsT = ones_f32[:] if rhs.dtype == DT.float32 else ones_bf[:]
        dps = ps_s.tile([P, 512], DT.float32, name="sps", tag="sps")
        nc.tensor.matmul(
            dps[:, 0:free],
            lhsT=lhsT,
            rhs=rhs,
            start=True,
            stop=True,
            skip_group_check=True,
        )

    # weight layout in SBUF: [128 (c within chunk), NCH*C (cchunk-major, d)]
    wq_bf = pool_const.tile([P, NCH * C], DT.bfloat16, name="wq_bf")
    wk_bf = pool_const.tile([P, NCH * C], DT.bfloat16, name="wk_bf")
    wv_bf = pool_const.tile([P, NCH * C], DT.bfloat16, name="wv_bf")
    for w_bf, w_d, wnm in ((wq_bf, wqt, "q"), (wk_bf, wkt, "k"), (wv_bf, wvt, "v")):
        wtmp = pool_wtmp.tile([P, NCH * C], DT.float32, name=f"wtmp_{wnm}")
        nc.gpsimd.dma_start(
            wtmp[:].rearrange("p (a d) -> p a d", a=NCH),
            w_d[:].rearrange("(a p) d -> p a d", p=P),
        )
        nc.vector.tensor_copy(w_bf[:], wtmp[:])

    # ~4.3us of back-to-back dummy matmuls at kernel start: push the HAM
    # through its cold window while the first DMAs are in flight.
    for _ in range(10):
        warm_mm()

    st = [dict() for _ in range(B_SH)]

    def prologue_io(b, fast):
        """DMA loads + f32->bf16 casts.  fast=True (batch 0, nothing else
        running) splits casts across DVE and ScalarE; otherwise they go to
        the idle GpSimd engine so DVE/ScalarE stay free for compute."""
        s = st[b]
        s["s_f"], s["t_f"], s["s_bf"], s["t_bf"] = [], [], [], []
        for cc in range(NCH):
            sf = pool_sf.tile([P, T], DT.float32, name=f"s_f{cc}")
            nc.gpsimd.dma_start(sf[:], src[b, cc * P : (cc + 1) * P, :])
            s["s_f"].append(sf)
            tf = pool_tf.tile([P, T], DT.float32, name=f"t_f{cc}")
            nc.sync.dma_start(tf[:], trg[b, cc * P : (cc + 1) * P, :])
            s["t_f"].append(tf)
        if fast:
            # pace a dummy matmul off the first DMA so the PE HAM gate
            # doesn't see a >3.4us idle window during the load phase
            warm_mm(s["s_f"][0][:, 0:128])
        for cc in range(NCH):
            sb = pool_sbf.tile([P, T], DT.bfloat16, name=f"s_bf{cc}")
            tb = pool_tbf.tile([P, T], DT.bfloat16, name=f"t_bf{cc}")
            # casts sliced in halves: consumers (subtile deps) start earlier
            # and each slice doubles as a HAM pacing tick on batch 0
            for hh in range(2):
                sl = slice(1024 * hh, 1024 * (hh + 1))
                if fast:
                    nc.vector.tensor_copy(sb[:, sl], s["s_f"][cc][:, sl])
                    warm_mm(sb[:, 1024 * hh : 1024 * hh + 512])
                    nc.scalar.copy(tb[:, sl], s["t_f"][cc][:, sl])
                    warm_mm(tb[:, 1024 * hh : 1024 * hh + 512])
                else:
                    nc.gpsimd.tensor_copy(sb[:, sl], s["s_f"][cc][:, sl])
                    nc.gpsimd.tensor_copy(tb[:, sl], s["t_f"][cc][:, sl])
            s["s_bf"].append(sb)
            s["t_bf"].append(tb)

    def rowstats(b, x_bf, nm):
        """-> (mean [P,1] f32 AP, inv_sd [P,1] f32 tile) per row over T."""
        bnst = pool_stat.tile([P, 4 * 6], DT.float32, name=f"bnst_{nm}")
        for j in range(4):
            nc.vector.bn_stats(bnst[:, 6 * j : 6 * (j + 1)], x_bf[:, 512 * j : 512 * (j + 1)])
        mv = pool_stat.tile([P, 2], DT.float32, name=f"mv_{nm}")
        nc.vector.bn_aggr(mv[:], bnst[:])
        if b == 0:
            warm_mm(bnst[:, 0:24])  # HAM pacing tick through the stats phase
        sd = pool_stat.tile([P, 1], DT.float32, name=f"sd_{nm}")
        # sd = sqrt(var_pop * T/(T-1)) + EPS
        nc.scalar.activation(sd[:], mv[:, 1:2], ACTF.Sqrt, scale=float(T) / (T - 1))
        sde = pool_stat.tile([P, 1], DT.float32, name=f"sde_{nm}")
        nc.vector.tensor_scalar_add(sde[:], sd[:], EPS)
        inv = pool_stat.tile([P, 1], DT.float32, name=f"inv_{nm}")
        nc.vector.reciprocal(inv[:], sde[:])
        return mv[:, 0:1], inv

    def prologue_stats(b):
        s = st[b]
        s["mean_s"], s["inv_s"], s["mean_t"], s["inv_t"] = [], [], [], []
        for cc in range(NCH):
            m, i = rowstats(b, s["s_bf"][cc], f"s{cc}")
            s["mean_s"].append(m)
            s["inv_s"].append(i)
        for cc in range(NCH):
            m, i = rowstats(b, s["t_bf"][cc], f"t{cc}")
            s["mean_t"].append(m)
            s["inv_t"].append(i)

        # wq_s[c, d] = wqt[c, d] * inv_s[c]  (bf16), same for wk_s with inv_t
        wq_s = pool_stat.tile([P, NCH * C], DT.bfloat16, name="wq_s")
        wk_s = pool_stat.tile([P, NCH * C], DT.bfloat16, name="wk_s")
        mi_s, mi_t = [], []
        for cc in range(NCH):
            nc.vector.tensor_scalar_mul(
                wq_s[:, cc * C : (cc + 1) * C], wq_bf[:, cc * C : (cc + 1) * C], s["inv_s"][cc][:]
            )
            nc.vector.tensor_scalar_mul(
                wk_s[:, cc * C : (cc + 1) * C], wk_bf[:, cc * C : (cc + 1) * C], s["inv_t"][cc][:]
            )
            mis = pool_stat2.tile([P, 1], DT.bfloat16, name=f"mi_s{cc}")
            nc.vector.tensor_tensor(mis[:], s["mean_s"][cc], s["inv_s"][cc][:], ALU.mult)
            mi_s.append(mis)
            mit = pool_stat2.tile([P, 1], DT.bfloat16, name=f"mi_t{cc}")
            nc.vector.tensor_tensor(mit[:], s["mean_t"][cc], s["inv_t"][cc][:], ALU.mult)
            mi_t.append(mit)
        s["wq_s"], s["wk_s"], s["mi_s"], s["mi_t"] = wq_s, wk_s, mi_s, mi_t

        # PE pre-touches: pull cross-engine operand-ready waits off the first
        # real matmuls (MM encoding allows at most 2 sync waits).  Batch 0
        # only: for batch 1 the scheduler hoists these into the middle of
        # batch 0's score stream where they head-of-line block the PE on the
        # slow gpsimd input casts (measured 14us stall).
        if b == 0:
            for ap in (s["s_bf"][0], s["s_bf"][1], s["t_bf"][0], s["t_bf"][1]):
                nc.tensor.ldweights(weights=ap[:, 0:P])
            for ap in (wq_s, wk_s):
                nc.tensor.ldweights(weights=ap[:, 0:P])
            for ap in (mi_s[0], mi_s[1], mi_t[0], mi_t[1]):
                nc.tensor.ldweights(weights=ap[:])

        # beta[d] = sum_c w_s[c,d] * (mu[c]*inv[c]); psum [P, NCH] (d-chunk cols)
        negb = []
        for w_s, mi, nm in ((wq_s, mi_s, "q"), (wk_s, mi_t, "k")):
            bps = ps_s.tile([P, NCH], DT.float32, name="sps", tag="sps")
            for dc in range(NCH):
                for cc in range(NCH):
                    nc.tensor.matmul(
                        bps[:, dc : dc + 1],
                        lhsT=w_s[:, cc * C + dc * P : cc * C + (dc + 1) * P],
                        rhs=mi[cc][:],
                        start=(cc == 0),
                        stop=(cc == NCH - 1),
                    )
            nb = pool_stat2.tile([P, NCH], DT.float32, name=f"negb_{nm}")
            nc.vector.tensor_scalar_mul(nb[:], bps[:], -1.0)
            negb.append(nb)
        s["negbq"], s["negbk"] = negb

    def proj_qk(b):
        # Qt/Kt: [d, t] bf16 (per d-chunk tiles), bias folded during eviction
        s = st[b]
        s["qt_bf"], s["kt_bf"] = [], []
        for w_s, nb, outk, nm in (
            (s["wq_s"], s["negbq"], "qt_bf", "qt"),
            (s["wk_s"], s["negbk"], "kt_bf", "kt"),
        ):
            x_bf = s["s_bf"] if nm == "qt" else s["t_bf"]
            for dc in range(NCH):
                ot = pool_qk.tile([P, T], DT.bfloat16, name=f"{nm}{dc}")
                for half in range(2):
                    pps = ps_s.tile([P, 1024], DT.float32, name="sps", tag="sps")
                    for cc in range(NCH):
                        for n4 in range(2):
                            nc.tensor.matmul(
                                pps[:, 512 * n4 : 512 * (n4 + 1)],
                                lhsT=w_s[:, cc * C + dc * P : cc * C + (dc + 1) * P],
                                rhs=x_bf[cc][:, 1024 * half + 512 * n4 : 1024 * half + 512 * (n4 + 1)],
                                start=(cc == 0),
                                stop=(cc == NCH - 1),
                            )
                    nc.scalar.activation(
                        ot[:, 1024 * half : 1024 * (half + 1)],
                        pps[:],
                        ACTF.Identity,
                        bias=nb[:, dc : dc + 1],
                        scale=1.0,
                    )
                s[outk].append(ot)

    def proj_v(b):
        # V_T: [t within chunk, tchunk-major d]  (v_bf[p, 256*j + d])
        s = st[b]
        v_bf = pool_v.tile([P, NTCH * C], DT.bfloat16, name="v_bf")
        v2_bf = pool_v.tile([P, NTCH * C], DT.bfloat16, name="v2_bf")
        for g in range(4):
            vps = ps_s.tile([P, 1024], DT.float32, name="sps", tag="sps")
            for j4 in range(4):
                j = 4 * g + j4
                for cc in range(NCH):
                    nc.tensor.matmul(
                        vps[:, 256 * j4 : 256 * (j4 + 1)],
                        lhsT=s["t_bf"][cc][:, P * j : P * (j + 1)],
                        rhs=wv_bf[:, cc * C : (cc + 1) * C],
                        start=(cc == 0),
                        stop=(cc == NCH - 1),
                    )
            nc.scalar.copy(v_bf[:, 1024 * g : 1024 * (g + 1)], vps[:])
        # v^2 on GpSimd: a same-tensor tensor_tensor measured 7.9us on DVE
        # (port conflict); GpSimd is idle here and off the critical path.
        nc.gpsimd.tensor_tensor(v2_bf[:], v_bf[:], v_bf[:], ALU.mult)
        s["v_bf"], s["v2_bf"] = v_bf, v2_bf

    def stats_init(b):
        s = st[b]
        s["sm_h"] = pool_stat.tile([P, 2 * NCH], DT.float32, name="sm_h")
        s["sm2_h"] = pool_stat.tile([P, 2 * NCH], DT.float32, name="sm2_h")
        s["a_uh"] = pool_stat.tile([P, 2 * NTCH], DT.float32, name="a_uh")
        s["p_t"] = [[], []]
        s["z_ps"] = [None, None]

    def scores_half(b, sh):
        """scores^T -> exp -> Z accumulation for s-half sh."""
        s = st[b]
        so = 1024 * sh
        z_ps = ps_z.tile([P, 1024], DT.float32, name="zav", tag="zav")
        s["z_ps"][sh] = z_ps
        for tch in range(NTCH):
            p = pool_p.tile([P, 1024], DT.bfloat16, name="p")
            sps = ps_s.tile([P, 1024], DT.float32, name="sps", tag="sps")
            for dc in range(NCH):
                for n2 in range(2):
                    nc.tensor.matmul(
                        sps[:, 512 * n2 : 512 * (n2 + 1)],
                        lhsT=s["kt_bf"][dc][:, P * tch : P * (tch + 1)],
                        rhs=s["qt_bf"][dc][:, so + 512 * n2 : so + 512 * (n2 + 1)],
                        start=(dc == 0),
                        stop=(dc == NCH - 1),
                    )
            nc.scalar.activation(p[:], sps[:], ACTF.Exp, scale=1.0 / 16.0)
            for n2 in range(2):
                nc.tensor.matmul(
                    z_ps[:, 512 * n2 : 512 * (n2 + 1)],
                    lhsT=ones_bf[:],
                    rhs=p[:, 512 * n2 : 512 * (n2 + 1)],
                    start=(tch == 0),
                    stop=(tch == NTCH - 1),
                    skip_group_check=True,
                )
            s["p_t"][sh].append(p)

    def av_half(b, sh):
        """normalize p by 1/Z (accumulating a_u) and contract attn @ v."""
        s = st[b]
        z_ps = s["z_ps"][sh]
        # NB: reciprocal_approx_fast (custom DVE op) fails neuronxcc codegen
        # on this toolchain ("ISA wrong length"), so use the stock op.
        zinv = pool_zi.tile([P, 1024], DT.float32, name="zinv")
        nc.vector.reciprocal(zinv[:], z_ps[:])

        for i, p in enumerate(s["p_t"][sh]):
            # out = (P * 1) * zinv (normalize in place); accum -> a_u half
            nc.vector.scalar_tensor_tensor(
                out=p[:],
                in0=p[:],
                scalar=1.0,
                in1=zinv[:],
                op0=ALU.mult,
                op1=ALU.mult,
                accum_out=s["a_uh"][:, NTCH * sh + i : NTCH * sh + i + 1],
            )

        for dc in range(NCH):
            avps = ps_s.tile([P, 1024], DT.float32, name="sps", tag="sps")
            for tch in range(NTCH):
                for n2 in range(2):
                    nc.tensor.matmul(
                        avps[:, 512 * n2 : 512 * (n2 + 1)],
                        lhsT=s["v_bf"][:, 256 * tch + P * dc : 256 * tch + P * (dc + 1)],
                        rhs=s["p_t"][sh][tch][:, 512 * n2 : 512 * (n2 + 1)],
                        start=(tch == 0),
                        stop=(tch == NTCH - 1),
                    )
            junk = pool_junk.tile([P, 1024], DT.bfloat16, name="junk")
            nc.scalar.activation(
                junk[:], avps[:], ACTF.Square,
                accum_out=s["sm2_h"][:, NCH * sh + dc : NCH * sh + dc + 1],
            )
            nc.vector.reduce_sum(
                s["sm_h"][:, NCH * sh + dc : NCH * sh + dc + 1], avps[:],
                axis=mybir.AxisListType.X,
            )

    def finals(b):
        s = st[b]
        # combine halves
        a_u = pool_stat.tile([P, NTCH], DT.float32, name="a_u")
        nc.vector.tensor_add(a_u[:], s["a_uh"][:, 0:NTCH], s["a_uh"][:, NTCH : 2 * NTCH])
        a_ub = pool_stat2.tile([P, NTCH], DT.bfloat16, name="a_ub")
        nc.vector.tensor_copy(a_ub[:], a_u[:])
        sm = pool_stat.tile([P, NCH], DT.float32, name="sm")
        nc.vector.tensor_add(sm[:], s["sm_h"][:, 0:NCH], s["sm_h"][:, NCH : 2 * NCH])
        sm2 = pool_stat.tile([P, NCH], DT.float32, name="sm2")
        nc.vector.tensor_add(sm2[:], s["sm2_h"][:, 0:NCH], s["sm2_h"][:, NCH : 2 * NCH])

        # attn@v^2 matvec (per-partition result)
        av2_ps = ps_s.tile([P, NCH], DT.float32, name="sps", tag="sps")
        for dc in range(NCH):
            for j in range(NTCH):
                nc.tensor.matmul(
                    av2_ps[:, dc : dc + 1],
                    lhsT=s["v2_bf"][:, 256 * j + P * dc : 256 * j + P * (dc + 1)],
                    rhs=a_ub[:, j : j + 1],
                    start=(j == 0),
                    stop=(j == NTCH - 1),
                )

        # ================= finals + output ================================
        for dc in range(NCH):
            d1 = pool_stat.tile([P, 1], DT.float32, name=f"d1_{dc}")
            nc.vector.tensor_tensor(d1[:], av2_ps[:, dc : dc + 1], sm2[:, dc : dc + 1], ALU.subtract)
            r1 = pool_stat.tile([P, 1], DT.float32, name=f"r1_{dc}")
            nc.vector.tensor_scalar_max(r1[:], d1[:], 0.0)
            stdv = pool_stat.tile([P, 1], DT.float32, name=f"std_{dc}")
            nc.scalar.activation(stdv[:], r1[:], ACTF.Sqrt, scale=1.0 / T)
            av = pool_stat.tile([P, 1], DT.float32, name=f"av_{dc}")
            nc.vector.tensor_tensor(av[:], stdv[:], s["inv_s"][dc][:], ALU.mult)
            musc = pool_stat.tile([P, 1], DT.float32, name=f"musc_{dc}")
            nc.vector.tensor_scalar_mul(musc[:], sm[:, dc : dc + 1], 1.0 / T)
            negms = pool_stat.tile([P, 1], DT.float32, name=f"negms_{dc}")
            nc.vector.tensor_scalar_mul(negms[:], s["mean_s"][dc], -1.0)
            bv = pool_stat.tile([P, 1], DT.float32, name=f"bv_{dc}")
            nc.vector.scalar_tensor_tensor(
                out=bv[:], in0=av[:], scalar=negms[:], in1=musc[:], op0=ALU.mult, op1=ALU.add
            )
            for half in range(2):
                o_sb = pool_out.tile([P, 1024], DT.float32, name="o_sb")
                nc.scalar.activation(
                    o_sb[:],
                    s["s_bf"][dc][:, 1024 * half : 1024 * (half + 1)],
                    ACTF.Identity,
                    bias=bv[:],
                    scale=av[:],
                )
                nc.sync.dma_start(
                    out[b, dc * P : (dc + 1) * P, 1024 * half : 1024 * (half + 1)], o_sb[:]
                )

    # ================= software-pipelined emission ======================
    prologue_io(0, fast=True)
    prologue_stats(0)
    stats_init(0)
    proj_qk(0)
    proj_v(0)
    scores_half(0, 0)
    prologue_io(1, fast=False)
    scores_half(0, 1)
    av_half(0, 0)
    prologue_stats(1)
    stats_init(1)
    proj_qk(1)
    av_half(0, 1)
    finals(0)
    # proj_v(1) must come after finals(0): v2_bf(1)'s DVE write reuses
    # v2_bf(0)'s slot, whose last reader is the av2 matvec in finals(0) --
    # and that matvec needs a_ub from DVE instructions that would otherwise
    # queue behind the v2 multiply.
    proj_v(1)
    scores_half(1, 0)
    scores_half(1, 1)
    av_half(1, 0)
    av_half(1, 1)
    finals(1)


_NC_CACHE = None


def _get_nc():
    global _NC_CACHE
    if _NC_CACHE is None:
        _NC_CACHE = _build_nc()
    return _NC_CACHE


def _run(src, trg, Wq, Wk, Wv, **kwargs):
    src = np.ascontiguousarray(np.asarray(src, dtype=np.float32))
    trg = np.ascontiguousarray(np.asarray(trg, dtype=np.float32))
    wqt = np.ascontiguousarray(np.asarray(Wq, dtype=np.float32).T)
    wkt = np.ascontiguousarray(np.asarray(Wk, dtype=np.float32).T)
    wvt = np.ascontiguousarray(np.asarray(Wv, dtype=np.float32).T)
    nc = _get_nc()
    in_maps = [
        {
            "src": src[i * B_SH : (i + 1) * B_SH],
            "trg": trg[i * B_SH : (i + 1) * B_SH],
            "wqt": wqt,
            "wkt": wkt,
            "wvt": wvt,
        }
        for i in range(N_CORES)
    ]
    res = run_bass_kernel_spmd(nc, in_maps, list(range(N_CORES)), **kwargs)
    outp = np.concatenate([res.results[i]["out"] for i in range(N_CORES)], axis=0)
    return outp.astype(np.float32), res


def kernel(src, trg, Wq, Wk, Wv):
    outp, _ = _run(src, trg, Wq, Wk, Wv)
    return outp


# revision 17
# speedup vs baseline: 1.0628x; 1.0111x over previous
"""Trainium2 Bass kernel for nn_ChannelAdaptiveNormalization.

Reference computation (per batch):
    src_n = instnorm(src); q = Wq@src_n; k = Wk@instnorm(trg); v = Wv@trg
    attn = softmax(q^T k / sqrt(C))  over t
    mean = attn @ v ; var = relu(attn @ v^2 - mean^2)
    out = sqrt(mean_s[var]) * src_n + mean_s[mean]      (broadcast over time)

Kernel decomposition (all per-core, data-parallel over batch, 2 batches/core):
  * instance-norm is folded into the CxC projection weights (scale columns by
    1/sd, subtract a rank-1 bias) -- normalized activations never materialize.
  * scores are produced TRANSPOSED ([t, s]) so the attn contraction over t
    needs no transposes; softmax uses exp without max subtraction (scores are
    ~N(0,1)); Z (softmax denominators) via a replicated ones-matmul.
  * only column-reductions of mean/var over s are needed, so the full
    mean matrix is reduced on the fly from PSUM; attn@v^2 collapses to
    a tiny matvec with a_u[t] = sum_s attn[t,s].
  * final output is a per-(b,c) affine of raw src: out = A*src + B.

Scheduling (v2): the PE HAM clock-gate runs the array at 1.2 GHz until it
sees ~3.4us of sustained matmul work, and re-throttles after any ~3.4us idle
gap.  The emission order therefore software-pipelines the two batches and
the two s-halves so the PE queue never stalls more than ~1us: scores of the
next half are emitted before the softmax-normalize/AV of the previous one,
and batch 1's projections fill the PE gap while batch 0's AV waits on the
DVE normalize.  Dummy matmuls paced through the (DMA-bound) prologue keep
the HAM warm before the first projection burst.
"""

import os
import sys

import numpy as np

if "/opt/trn_rl_repo" not in sys.path:
    sys.path.insert(0, "/opt/trn_rl_repo")

from contextlib import ExitStack

import concourse.bass as bass
import concourse.tile as tile
from concourse import mybir
from concourse.bass_utils import run_bass_kernel_spmd

DT = mybir.dt
ALU = mybir.AluOpType
ACTF = mybir.ActivationFunctionType

N_CORES = 8
B_FULL = 16
B_SH = B_FULL // N_CORES  # 2 batches per core
C = 256
T = 2048
P = 128
NCH = C // P  # 2 channel chunks
NTCH = T // P  # 16 time chunks
EPS = 1e-5


def _build_nc() -> bass.Bass:
    nc = bass.Bass()
    src = nc.declare_dram_parameter("src", [B_SH, C, T], DT.float32, isOutput=False)
    trg = nc.declare_dram_parameter("trg", [B_SH, C, T], DT.float32, isOutput=False)
    wqt = nc.declare_dram_parameter("wqt", [C, C], DT.float32, isOutput=False)
    wkt = nc.declare_dram_parameter("wkt", [C, C], DT.float32, isOutput=False)
    wvt = nc.declare_dram_parameter("wvt", [C, C], DT.float32, isOutput=False)
    out = nc.declare_dram_parameter("out", [B_SH, C, T], DT.float32, isOutput=True)

    with tile.TileContext(nc) as tc:
        with ExitStack() as ctx:
            _build_kernel(ctx, tc, src, trg, wqt, wkt, wvt, out)
    _legalize_waits(nc)
    return nc


def _legalize_waits(nc: bass.Bass):
    """walrus on this toolchain encodes at most ONE sync wait per
    instruction (NEURON_ISA_TPB_EVENTS has a single wait slot and no
    splitting pass runs).  Hoist all but the last wait of every
    instruction into standalone single-wait EventSemaphore instructions
    on the same engine queue, which preserves ordering semantics."""
    # collect all tile-context data semaphores (skip barrier sems)
    all_sems = {}
    for fn in nc.m.functions:
        for blk in fn.blocks:
            for inst in blk.instructions:
                si = getattr(inst, "sync_info", None)
                if si is None:
                    continue
                for w in list(si.on_wait) + list(si.on_update):
                    if not w.ant_name.startswith("barrier"):
                        all_sems[w.id] = w.ant_name

    for fn in nc.m.functions:
        for blk in fn.blocks:
            snapshot = list(blk.instructions)
            for idx in range(len(snapshot) - 1, -1, -1):
                inst = snapshot[idx]
                if type(inst).__name__ == "InstISA" and getattr(inst, "isa_opcode", None) == 176:
                    # EVENT_SEMAPHORE_RANGE_CLEAR: encoding mismatches this
                    # walrus build; replace with per-sem zero-writes.
                    pos = list(blk.instructions).index(inst)
                    blk.instructions.pop(pos)
                    for sid, sname in sorted(all_sems.items()):
                        ev = mybir.InstEventSemaphore(
                            name=nc.get_next_instruction_name(), ins=[], outs=[]
                        )
                        ev.engine = inst.engine
                        ev.sync_info = mybir.SyncInfo(
                            on_wait=[],
                            on_update=[
                                mybir.SyncUpdate(
                                    sync_type="semaphore",
                                    id=sid,
                                    ant_name=sname,
                                    update_mode="sem-wr-imm",
                                    update_value=0,
                                )
                            ],
                        )
                        nc.register_instruction(ev)
                        blk.instructions.insert(pos, ev)
                        pos += 1

    for fn in nc.m.functions:
        for blk in fn.blocks:
            snapshot = list(blk.instructions)
            for idx in range(len(snapshot) - 1, -1, -1):
                inst = snapshot[idx]
                si = getattr(inst, "sync_info", None)
                if si is None or len(si.on_wait) <= 1:
                    continue
                waits = list(si.on_wait)
                evs = []
                for w in waits[:-1]:
                    ev = mybir.InstEventSemaphore(
                        name=nc.get_next_instruction_name(), ins=[], outs=[]
                    )
                    ev.engine = inst.engine
                    ev.sync_info = mybir.SyncInfo(on_wait=[w], on_update=[])
                    nc.register_instruction(ev)
                    evs.append(ev)
                si.on_wait = waits[-1:]
                inst.sync_info = si
                for ev in reversed(evs):
                    blk.instructions.insert(idx, ev)


def _build_kernel(ctx, tc, src, trg, wqt, wkt, wvt, out):
    nc = tc.nc
    ep = ctx.enter_context

    pool_const = ep(tc.tile_pool(name="const", bufs=1))
    pool_wtmp = ep(tc.tile_pool(name="wtmp", bufs=1))
    pool_sf = ep(tc.tile_pool(name="sf", bufs=1))
    pool_tf = ep(tc.tile_pool(name="tf", bufs=1))
    pool_sbf = ep(tc.tile_pool(name="sbf", bufs=2))
    pool_tbf = ep(tc.tile_pool(name="tbf", bufs=2))
    pool_qk = ep(tc.tile_pool(name="qk", bufs=1))
    pool_v = ep(tc.tile_pool(name="vpool", bufs=1))
    # 32 bufs: both s-halves' p tiles must be live simultaneously, since
    # half 1's scores are emitted before half 0's AV (a smaller ring makes
    # half-1 exp wait on half-0 AV matmuls that sit *behind* half-1's Z
    # matmuls in the in-order PE queue -> deadlock).
    pool_p = ep(tc.tile_pool(name="ppool", bufs=32))
    pool_zi = ep(tc.tile_pool(name="zipool", bufs=2))
    pool_stat = ep(tc.tile_pool(name="stat", bufs=2))
    pool_stat2 = ep(tc.tile_pool(name="stat2", bufs=2))
    pool_junk = ep(tc.tile_pool(name="junk", bufs=2))
    pool_out = ep(tc.tile_pool(name="outio", bufs=2))
    ps_s = ep(tc.tile_pool(name="ps_s", bufs=2, space="PSUM"))
    ps_z = ep(tc.tile_pool(name="ps_z", bufs=2, space="PSUM"))

    # ---- constants / weights (once) ----
    ones_bf = pool_const.tile([P, P], DT.bfloat16, name="ones_bf")
    nc.vector.memset(ones_bf[:], 1.0)
    dummy_bf = pool_const.tile([P, 512], DT.bfloat16, name="dummy_bf")
    nc.vector.memset(dummy_bf[:], 0.0)
    ones_f32 = pool_const.tile([P, P], DT.float32, name="ones_f32")
    nc.vector.memset(ones_f32[:], 1.0)

    def warm_mm(rhs_ap=None):
        """One dummy matmul to keep the PE HAM clock-gate warm.  If rhs_ap
        is given the matmul waits for it, pacing the dummy to just after
        that producer; otherwise it runs as soon as the PE is free."""
        rhs = rhs_ap if rhs_ap is not None else dummy_bf[:]
        free = rhs.shape[-1]
        lhsT = ones_f32[:] if rhs.dtype == DT.float32 else ones_bf[:]
        dps = ps_s.tile([P, 512], DT.float32, name="sps", tag="sps")
        nc.tensor.matmul(
            dps[:, 0:free],
            lhsT=lhsT,
            rhs=rhs,
            start=True,
            stop=True,
            skip_group_check=True,
        )

    # weight layout in SBUF: [128 (c within chunk), NCH*C (cchunk-major, d)]
    wq_bf = pool_const.tile([P, NCH * C], DT.bfloat16, name="wq_bf")
    wk_bf = pool_const.tile([P, NCH * C], DT.bfloat16, name="wk_bf")
    wv_bf = pool_const.tile([P, NCH * C], DT.bfloat16, name="wv_bf")
    for w_bf, w_d, wnm in ((wq_bf, wqt, "q"), (wk_bf, wkt, "k"), (wv_bf, wvt, "v")):
        wtmp = pool_wtmp.tile([P, NCH * C], DT.float32, name=f"wtmp_{wnm}")
        nc.gpsimd.dma_start(
            wtmp[:].rearrange("p (a d) -> p a d", a=NCH),
            w_d[:].rearrange("(a p) d -> p a d", p=P),
        )
        nc.vector.tensor_copy(w_bf[:], wtmp[:])

    # ~4.3us of back-to-back dummy matmuls at kernel start: push the HAM
    # through its cold window while the first DMAs are in flight.
    for _ in range(10):
        warm_mm()

    st = [dict() for _ in range(B_SH)]

    def prologue_io(b, fast):
        """DMA loads + f32->bf16 casts.  fast=True (batch 0, nothing else
        running) splits casts across DVE and ScalarE; otherwise they go to
        the idle GpSimd engine so DVE/ScalarE stay free for compute."""
        s = st[b]
        s["s_f"], s["t_f"], s["s_bf"], s["t_bf"] = [], [], [], []
        for cc in range(NCH):
            sf = pool_sf.tile([P, T], DT.float32, name=f"s_f{cc}")
            nc.gpsimd.dma_start(sf[:], src[b, cc * P : (cc + 1) * P, :])
            s["s_f"].append(sf)
            tf = pool_tf.tile([P, T], DT.float32, name=f"t_f{cc}")
            nc.sync.dma_start(tf[:], trg[b, cc * P : (cc + 1) * P, :])
            s["t_f"].append(tf)
        if fast:
            # pace a dummy matmul off the first DMA so the PE HAM gate
            # doesn't see a >3.4us idle window during the load phase
            warm_mm(s["s_f"][0][:, 0:128])
        for cc in range(NCH):
            sb = pool_sbf.tile([P, T], DT.bfloat16, name=f"s_bf{cc}")
            tb = pool_tbf.tile([P, T], DT.bfloat16, name=f"t_bf{cc}")
            # casts sliced in halves: consumers (subtile deps) start earlier
            # and each slice doubles as a HAM pacing tick on batch 0
            for hh in range(2):
                sl = slice(1024 * hh, 1024 * (hh + 1))
                if fast:
                    nc.vector.tensor_copy(sb[:, sl], s["s_f"][cc][:, sl])
                    warm_mm(sb[:, 1024 * hh : 1024 * hh + 512])
                    nc.scalar.copy(tb[:, sl], s["t_f"][cc][:, sl])
                    warm_mm(tb[:, 1024 * hh : 1024 * hh + 512])
                else:
                    nc.gpsimd.tensor_copy(sb[:, sl], s["s_f"][cc][:, sl])
                    nc.gpsimd.tensor_copy(tb[:, sl], s["t_f"][cc][:, sl])
            s["s_bf"].append(sb)
            s["t_bf"].append(tb)

    def rowstats(b, x_bf, nm):
        """-> (mean [P,1] f32 AP, inv_sd [P,1] f32 tile) per row over T."""
        bnst = pool_stat.tile([P, 4 * 6], DT.float32, name=f"bnst_{nm}")
        for j in range(4):
            nc.vector.bn_stats(bnst[:, 6 * j : 6 * (j + 1)], x_bf[:, 512 * j : 512 * (j + 1)])
        mv = pool_stat.tile([P, 2], DT.float32, name=f"mv_{nm}")
        nc.vector.bn_aggr(mv[:], bnst[:])
        if b == 0:
            warm_mm(bnst[:, 0:24])  # HAM pacing tick through the stats phase
        sd = pool_stat.tile([P, 1], DT.float32, name=f"sd_{nm}")
        # sd = sqrt(var_pop * T/(T-1)) + EPS
        nc.scalar.activation(sd[:], mv[:, 1:2], ACTF.Sqrt, scale=float(T) / (T - 1))
        sde = pool_stat.tile([P, 1], DT.float32, name=f"sde_{nm}")
        nc.vector.tensor_scalar_add(sde[:], sd[:], EPS)
        inv = pool_stat.tile([P, 1], DT.float32, name=f"inv_{nm}")
        nc.vector.reciprocal(inv[:], sde[:])
        return mv[:, 0:1], inv

    def prologue_stats(b):
        s = st[b]
        s["mean_s"], s["inv_s"], s["mean_t"], s["inv_t"] = [], [], [], []
        for cc in range(NCH):
            m, i = rowstats(b, s["s_bf"][cc], f"s{cc}")
            s["mean_s"].append(m)
            s["inv_s"].append(i)
        for cc in range(NCH):
            m, i = rowstats(b, s["t_bf"][cc], f"t{cc}")
            s["mean_t"].append(m)
            s["inv_t"].append(i)

        # wq_s[c, d] = wqt[c, d] * inv_s[c]  (bf16), same for wk_s with inv_t
        # weight scaling on ScalarE (per-partition scale AP): DVE
        # tensor_scalar with an AP scalar measured 2.4us per chunk and sat on
        # the critical path to the first projection.
        wq_s = pool_stat.tile([P, NCH * C], DT.bfloat16, name="wq_s")
        wk_s = pool_stat.tile([P, NCH * C], DT.bfloat16, name="wk_s")
        mi_s, mi_t = [], []
        for cc in range(NCH):
            nc.scalar.activation(
                wq_s[:, cc * C : (cc + 1) * C], wq_bf[:, cc * C : (cc + 1) * C],
                ACTF.Identity, scale=s["inv_s"][cc][:],
            )
            nc.scalar.activation(
                wk_s[:, cc * C : (cc + 1) * C], wk_bf[:, cc * C : (cc + 1) * C],
                ACTF.Identity, scale=s["inv_t"][cc][:],
            )
            mis = pool_stat2.tile([P, 1], DT.bfloat16, name=f"mi_s{cc}")
            nc.vector.tensor_tensor(mis[:], s["mean_s"][cc], s["inv_s"][cc][:], ALU.mult)
            mi_s.append(mis)
            mit = pool_stat2.tile([P, 1], DT.bfloat16, name=f"mi_t{cc}")
            nc.vector.tensor_tensor(mit[:], s["mean_t"][cc], s["inv_t"][cc][:], ALU.mult)
            mi_t.append(mit)
        s["wq_s"], s["wk_s"], s["mi_s"], s["mi_t"] = wq_s, wk_s, mi_s, mi_t

        # PE pre-touches: pull cross-engine operand-ready waits off the first
        # real matmuls (MM encoding allows at most 2 sync waits).  Batch 0
        # only: for batch 1 the scheduler hoists these into the middle of
        # batch 0's score stream where they head-of-line block the PE on the
        # slow gpsimd input casts (measured 14us stall).
        if b == 0:
            for ap in (s["s_bf"][0], s["s_bf"][1], s["t_bf"][0], s["t_bf"][1]):
                nc.tensor.ldweights(weights=ap[:, 0:P])
            for ap in (wq_s, wk_s):
                nc.tensor.ldweights(weights=ap[:, 0:P])
            for ap in (mi_s[0], mi_s[1], mi_t[0], mi_t[1]):
                nc.tensor.ldweights(weights=ap[:])

        # beta[d] = sum_c w_s[c,d] * (mu[c]*inv[c]); psum [P, NCH] (d-chunk cols)
        negb = []
        for w_s, mi, nm in ((wq_s, mi_s, "q"), (wk_s, mi_t, "k")):
            bps = ps_s.tile([P, NCH], DT.float32, name="sps", tag="sps")
            for dc in range(NCH):
                for cc in range(NCH):
                    nc.tensor.matmul(
                        bps[:, dc : dc + 1],
                        lhsT=w_s[:, cc * C + dc * P : cc * C + (dc + 1) * P],
                        rhs=mi[cc][:],
                        start=(cc == 0),
                        stop=(cc == NCH - 1),
                    )
            nb = pool_stat2.tile([P, NCH], DT.float32, name=f"negb_{nm}")
            nc.vector.tensor_scalar_mul(nb[:], bps[:], -1.0)
            negb.append(nb)
        s["negbq"], s["negbk"] = negb

    def proj_qk(b):
        # Qt/Kt: [d, t] bf16 (per d-chunk tiles), bias folded during eviction
        s = st[b]
        s["qt_bf"], s["kt_bf"] = [], []
        for w_s, nb, outk, nm in (
            (s["wq_s"], s["negbq"], "qt_bf", "qt"),
            (s["wk_s"], s["negbk"], "kt_bf", "kt"),
        ):
            x_bf = s["s_bf"] if nm == "qt" else s["t_bf"]
            for dc in range(NCH):
                ot = pool_qk.tile([P, T], DT.bfloat16, name=f"{nm}{dc}")
                for half in range(2):
                    pps = ps_s.tile([P, 1024], DT.float32, name="sps", tag="sps")
                    for cc in range(NCH):
                        for n4 in range(2):
                            nc.tensor.matmul(
                                pps[:, 512 * n4 : 512 * (n4 + 1)],
                                lhsT=w_s[:, cc * C + dc * P : cc * C + (dc + 1) * P],
                                rhs=x_bf[cc][:, 1024 * half + 512 * n4 : 1024 * half + 512 * (n4 + 1)],
                                start=(cc == 0),
                                stop=(cc == NCH - 1),
                            )
                    nc.scalar.activation(
                        ot[:, 1024 * half : 1024 * (half + 1)],
                        pps[:],
                        ACTF.Identity,
                        bias=nb[:, dc : dc + 1],
                        scale=1.0,
                    )
                s[outk].append(ot)

    def proj_v(b):
        # V_T: [t within chunk, tchunk-major d]  (v_bf[p, 256*j + d])
        s = st[b]
        v_bf = pool_v.tile([P, NTCH * C], DT.bfloat16, name="v_bf")
        v2_bf = pool_v.tile([P, NTCH * C], DT.bfloat16, name="v2_bf")
        for g in range(4):
            vps = ps_s.tile([P, 1024], DT.float32, name="sps", tag="sps")
            for j4 in range(4):
                j = 4 * g + j4
                for cc in range(NCH):
                    nc.tensor.matmul(
                        vps[:, 256 * j4 : 256 * (j4 + 1)],
                        lhsT=s["t_bf"][cc][:, P * j : P * (j + 1)],
                        rhs=wv_bf[:, cc * C : (cc + 1) * C],
                        start=(cc == 0),
                        stop=(cc == NCH - 1),
                    )
            nc.scalar.copy(v_bf[:, 1024 * g : 1024 * (g + 1)], vps[:])
        # v^2 on GpSimd: a same-tensor tensor_tensor measured 7.9us on DVE
        # (port conflict); GpSimd is idle here and off the critical path.
        nc.gpsimd.tensor_tensor(v2_bf[:], v_bf[:], v_bf[:], ALU.mult)
        s["v_bf"], s["v2_bf"] = v_bf, v2_bf

    def stats_init(b):
        s = st[b]
        s["sm_h"] = pool_stat.tile([P, 2 * NCH], DT.float32, name="sm_h")
        s["sm2_h"] = pool_stat.tile([P, 2 * NCH], DT.float32, name="sm2_h")
        s["a_uh"] = pool_stat.tile([P, 2 * NTCH], DT.float32, name="a_uh")
        s["p_t"] = [[], []]
        s["z_ps"] = [None, None]

    def scores_half(b, sh):
        """scores^T -> exp -> Z accumulation for s-half sh."""
        s = st[b]
        so = 1024 * sh
        z_ps = ps_z.tile([P, 1024], DT.float32, name="zav", tag="zav")
        s["z_ps"][sh] = z_ps
        for tch in range(NTCH):
            p = pool_p.tile([P, 1024], DT.bfloat16, name="p")
            sps = ps_s.tile([P, 1024], DT.float32, name="sps", tag="sps")
            for dc in range(NCH):
                for n2 in range(2):
                    nc.tensor.matmul(
                        sps[:, 512 * n2 : 512 * (n2 + 1)],
                        lhsT=s["kt_bf"][dc][:, P * tch : P * (tch + 1)],
                        rhs=s["qt_bf"][dc][:, so + 512 * n2 : so + 512 * (n2 + 1)],
                        start=(dc == 0),
                        stop=(dc == NCH - 1),
                    )
            nc.scalar.activation(p[:], sps[:], ACTF.Exp, scale=1.0 / 16.0)
            for n2 in range(2):
                nc.tensor.matmul(
                    z_ps[:, 512 * n2 : 512 * (n2 + 1)],
                    lhsT=ones_bf[:],
                    rhs=p[:, 512 * n2 : 512 * (n2 + 1)],
                    start=(tch == 0),
                    stop=(tch == NTCH - 1),
                    skip_group_check=True,
                )
            s["p_t"][sh].append(p)

    def av_half(b, sh):
        """normalize p by 1/Z (accumulating a_u) and contract attn @ v."""
        s = st[b]
        z_ps = s["z_ps"][sh]
        # 1/Z as exp(-ln Z) on ScalarE: DVE reciprocal measured 6.7us per
        # [128,1024] tile and gated every AV phase; two ACT passes cost ~2us
        # on the less-loaded scalar queue.  (reciprocal_approx_fast, the
        # custom DVE op, fails neuronxcc codegen on this toolchain.)
        zln = pool_zi.tile([P, 1024], DT.float32, name="zln")
        nc.scalar.activation(zln[:], z_ps[:], ACTF.Ln)
        zinv = pool_zi.tile([P, 1024], DT.float32, name="zinv")
        nc.scalar.activation(zinv[:], zln[:], ACTF.Exp, scale=-1.0)

        for i, p in enumerate(s["p_t"][sh]):
            # out = (P * 1) * zinv (normalize in place); accum -> a_u half
            nc.vector.scalar_tensor_tensor(
                out=p[:],
                in0=p[:],
                scalar=1.0,
                in1=zinv[:],
                op0=ALU.mult,
                op1=ALU.mult,
                accum_out=s["a_uh"][:, NTCH * sh + i : NTCH * sh + i + 1],
            )

        for dc in range(NCH):
            avps = ps_s.tile([P, 1024], DT.float32, name="sps", tag="sps")
            for tch in range(NTCH):
                for n2 in range(2):
                    nc.tensor.matmul(
                        avps[:, 512 * n2 : 512 * (n2 + 1)],
                        lhsT=s["v_bf"][:, 256 * tch + P * dc : 256 * tch + P * (dc + 1)],
                        rhs=s["p_t"][sh][tch][:, 512 * n2 : 512 * (n2 + 1)],
                        start=(tch == 0),
                        stop=(tch == NTCH - 1),
                    )
            junk = pool_junk.tile([P, 1024], DT.bfloat16, name="junk")
            nc.scalar.activation(
                junk[:], avps[:], ACTF.Square,
                accum_out=s["sm2_h"][:, NCH * sh + dc : NCH * sh + dc + 1],
            )
            nc.vector.reduce_sum(
                s["sm_h"][:, NCH * sh + dc : NCH * sh + dc + 1], avps[:],
                axis=mybir.AxisListType.X,
            )

    def finals(b):
        s = st[b]
        # combine halves
        a_u = pool_stat.tile([P, NTCH], DT.float32, name="a_u")
        nc.vector.tensor_add(a_u[:], s["a_uh"][:, 0:NTCH], s["a_uh"][:, NTCH : 2 * NTCH])
        a_ub = pool_stat2.tile([P, NTCH], DT.bfloat16, name="a_ub")
        nc.vector.tensor_copy(a_ub[:], a_u[:])
        sm = pool_stat.tile([P, NCH], DT.float32, name="sm")
        nc.vector.tensor_add(sm[:], s["sm_h"][:, 0:NCH], s["sm_h"][:, NCH : 2 * NCH])
        sm2 = pool_stat.tile([P, NCH], DT.float32, name="sm2")
        nc.vector.tensor_add(sm2[:], s["sm2_h"][:, 0:NCH], s["sm2_h"][:, NCH : 2 * NCH])

        # attn@v^2 matvec (per-partition result)
        av2_ps = ps_s.tile([P, NCH], DT.float32, name="sps", tag="sps")
        for dc in range(NCH):
            for j in range(NTCH):
                nc.tensor.matmul(
                    av2_ps[:, dc : dc + 1],
                    lhsT=s["v2_bf"][:, 256 * j + P * dc : 256 * j + P * (dc + 1)],
                    rhs=a_ub[:, j : j + 1],
                    start=(j == 0),
                    stop=(j == NTCH - 1),
                )

        # ================= finals + output ================================
        for dc in range(NCH):
            d1 = pool_stat.tile([P, 1], DT.float32, name=f"d1_{dc}")
            nc.vector.tensor_tensor(d1[:], av2_ps[:, dc : dc + 1], sm2[:, dc : dc + 1], ALU.subtract)
            r1 = pool_stat.tile([P, 1], DT.float32, name=f"r1_{dc}")
            nc.vector.tensor_scalar_max(r1[:], d1[:], 0.0)
            stdv = pool_stat.tile([P, 1], DT.float32, name=f"std_{dc}")
            nc.scalar.activation(stdv[:], r1[:], ACTF.Sqrt, scale=1.0 / T)
            av = pool_stat.tile([P, 1], DT.float32, name=f"av_{dc}")
            nc.vector.tensor_tensor(av[:], stdv[:], s["inv_s"][dc][:], ALU.mult)
            musc = pool_stat.tile([P, 1], DT.float32, name=f"musc_{dc}")
            nc.vector.tensor_scalar_mul(musc[:], sm[:, dc : dc + 1], 1.0 / T)
            negms = pool_stat.tile([P, 1], DT.float32, name=f"negms_{dc}")
            nc.vector.tensor_scalar_mul(negms[:], s["mean_s"][dc], -1.0)
            bv = pool_stat.tile([P, 1], DT.float32, name=f"bv_{dc}")
            nc.vector.scalar_tensor_tensor(
                out=bv[:], in0=av[:], scalar=negms[:], in1=musc[:], op0=ALU.mult, op1=ALU.add
            )
            for half in range(2):
                o_sb = pool_out.tile([P, 1024], DT.float32, name="o_sb")
                nc.scalar.activation(
                    o_sb[:],
                    s["s_bf"][dc][:, 1024 * half : 1024 * (half + 1)],
                    ACTF.Identity,
                    bias=bv[:],
                    scale=av[:],
                )
                nc.sync.dma_start(
                    out[b, dc * P : (dc + 1) * P, 1024 * half : 1024 * (half + 1)], o_sb[:]
                )

    # ================= software-pipelined emission ======================
    prologue_io(0, fast=True)
    proj_v(0)  # V needs no stats: fills the PE while DVE computes stats
    prologue_stats(0)
    stats_init(0)
    proj_qk(0)
    scores_half(0, 0)
    prologue_io(1, fast=False)
    scores_half(0, 1)
    av_half(0, 0)
    prologue_stats(1)
    stats_init(1)
    proj_qk(1)
    av_half(0, 1)
    finals(0)
    # proj_v(1) must come after finals(0): v2_bf(1)'s DVE write reuses
    # v2_bf(0)'s slot, whose last reader is the av2 matvec in finals(0) --
    # and that matvec needs a_ub from DVE instructions that would otherwise
    # queue behind the v2 multiply.
    proj_v(1)
    scores_half(1, 0)
    scores_half(1, 1)
    av_half(1, 0)
    av_half(1, 1)
    finals(1)


_NC_CACHE = None


def _get_nc():
    global _NC_CACHE
    if _NC_CACHE is None:
        _NC_CACHE = _build_nc()
    return _NC_CACHE


def _run(src, trg, Wq, Wk, Wv, **kwargs):
    src = np.ascontiguousarray(np.asarray(src, dtype=np.float32))
    trg = np.ascontiguousarray(np.asarray(trg, dtype=np.float32))
    wqt = np.ascontiguousarray(np.asarray(Wq, dtype=np.float32).T)
    wkt = np.ascontiguousarray(np.asarray(Wk, dtype=np.float32).T)
    wvt = np.ascontiguousarray(np.asarray(Wv, dtype=np.float32).T)
    nc = _get_nc()
    in_maps = [
        {
            "src": src[i * B_SH : (i + 1) * B_SH],
            "trg": trg[i * B_SH : (i + 1) * B_SH],
            "wqt": wqt,
            "wkt": wkt,
            "wvt": wvt,
        }
        for i in range(N_CORES)
    ]
    res = run_bass_kernel_spmd(nc, in_maps, list(range(N_CORES)), **kwargs)
    outp = np.concatenate([res.results[i]["out"] for i in range(N_CORES)], axis=0)
    return outp.astype(np.float32), res


def kernel(src, trg, Wq, Wk, Wv):
    outp, _ = _run(src, trg, Wq, Wk, Wv)
    return outp


# revision 22
# speedup vs baseline: 1.1266x; 1.0600x over previous
"""Trainium2 Bass kernel for nn_ChannelAdaptiveNormalization.

Reference computation (per batch):
    src_n = instnorm(src); q = Wq@src_n; k = Wk@instnorm(trg); v = Wv@trg
    attn = softmax(q^T k / sqrt(C))  over t
    mean = attn @ v ; var = relu(attn @ v^2 - mean^2)
    out = sqrt(mean_s[var]) * src_n + mean_s[mean]      (broadcast over time)

Kernel decomposition (all per-core, data-parallel over batch, 2 batches/core):
  * instance-norm is folded into the CxC projection weights (scale columns by
    1/sd, subtract a rank-1 bias) -- normalized activations never materialize.
  * scores are produced TRANSPOSED ([t, s]) so the attn contraction over t
    needs no transposes; softmax uses exp without max subtraction (scores are
    ~N(0,1)); Z (softmax denominators) via a replicated ones-matmul.
  * only column-reductions of mean/var over s are needed, so the full
    mean matrix is reduced on the fly from PSUM; attn@v^2 collapses to
    a tiny matvec with a_u[t] = sum_s attn[t,s].
  * final output is a per-(b,c) affine of raw src: out = A*src + B.

Scheduling (v2): the PE HAM clock-gate runs the array at 1.2 GHz until it
sees ~3.4us of sustained matmul work, and re-throttles after any ~3.4us idle
gap.  The emission order therefore software-pipelines the two batches and
the two s-halves so the PE queue never stalls more than ~1us: scores of the
next half are emitted before the softmax-normalize/AV of the previous one,
and batch 1's projections fill the PE gap while batch 0's AV waits on the
DVE normalize.  Dummy matmuls paced through the (DMA-bound) prologue keep
the HAM warm before the first projection burst.
"""

import os
import sys

import numpy as np

if "/opt/trn_rl_repo" not in sys.path:
    sys.path.insert(0, "/opt/trn_rl_repo")

from contextlib import ExitStack

import concourse.bass as bass
import concourse.tile as tile
from concourse import mybir
from concourse.bass_utils import run_bass_kernel_spmd

DT = mybir.dt
ALU = mybir.AluOpType
ACTF = mybir.ActivationFunctionType

N_CORES = 8
B_FULL = 16
B_SH = B_FULL // N_CORES  # 2 batches per core
C = 256
T = 2048
P = 128
NCH = C // P  # 2 channel chunks
NTCH = T // P  # 16 time chunks
EPS = 1e-5


def _build_nc() -> bass.Bass:
    nc = bass.Bass()
    src = nc.declare_dram_parameter("src", [B_SH, C, T], DT.float32, isOutput=False)
    trg = nc.declare_dram_parameter("trg", [B_SH, C, T], DT.float32, isOutput=False)
    wqt = nc.declare_dram_parameter("wqt", [C, C], DT.float32, isOutput=False)
    wkt = nc.declare_dram_parameter("wkt", [C, C], DT.float32, isOutput=False)
    wvt = nc.declare_dram_parameter("wvt", [C, C], DT.float32, isOutput=False)
    out = nc.declare_dram_parameter("out", [B_SH, C, T], DT.float32, isOutput=True)

    with tile.TileContext(nc) as tc:
        with ExitStack() as ctx:
            _build_kernel(ctx, tc, src, trg, wqt, wkt, wvt, out)
    _legalize_waits(nc)
    return nc


def _legalize_waits(nc: bass.Bass):
    """walrus on this toolchain encodes at most ONE sync wait per
    instruction (NEURON_ISA_TPB_EVENTS has a single wait slot and no
    splitting pass runs).  Hoist all but the last wait of every
    instruction into standalone single-wait EventSemaphore instructions
    on the same engine queue, which preserves ordering semantics."""
    # collect all tile-context data semaphores (skip barrier sems)
    all_sems = {}
    for fn in nc.m.functions:
        for blk in fn.blocks:
            for inst in blk.instructions:
                si = getattr(inst, "sync_info", None)
                if si is None:
                    continue
                for w in list(si.on_wait) + list(si.on_update):
                    if not w.ant_name.startswith("barrier"):
                        all_sems[w.id] = w.ant_name

    for fn in nc.m.functions:
        for blk in fn.blocks:
            snapshot = list(blk.instructions)
            for idx in range(len(snapshot) - 1, -1, -1):
                inst = snapshot[idx]
                if type(inst).__name__ == "InstISA" and getattr(inst, "isa_opcode", None) == 176:
                    # EVENT_SEMAPHORE_RANGE_CLEAR: encoding mismatches this
                    # walrus build; replace with per-sem zero-writes.
                    pos = list(blk.instructions).index(inst)
                    blk.instructions.pop(pos)
                    for sid, sname in sorted(all_sems.items()):
                        ev = mybir.InstEventSemaphore(
                            name=nc.get_next_instruction_name(), ins=[], outs=[]
                        )
                        ev.engine = inst.engine
                        ev.sync_info = mybir.SyncInfo(
                            on_wait=[],
                            on_update=[
                                mybir.SyncUpdate(
                                    sync_type="semaphore",
                                    id=sid,
                                    ant_name=sname,
                                    update_mode="sem-wr-imm",
                                    update_value=0,
                                )
                            ],
                        )
                        nc.register_instruction(ev)
                        blk.instructions.insert(pos, ev)
                        pos += 1

    for fn in nc.m.functions:
        for blk in fn.blocks:
            snapshot = list(blk.instructions)
            for idx in range(len(snapshot) - 1, -1, -1):
                inst = snapshot[idx]
                si = getattr(inst, "sync_info", None)
                if si is None or len(si.on_wait) <= 1:
                    continue
                waits = list(si.on_wait)
                evs = []
                for w in waits[:-1]:
                    ev = mybir.InstEventSemaphore(
                        name=nc.get_next_instruction_name(), ins=[], outs=[]
                    )
                    ev.engine = inst.engine
                    ev.sync_info = mybir.SyncInfo(on_wait=[w], on_update=[])
                    nc.register_instruction(ev)
                    evs.append(ev)
                si.on_wait = waits[-1:]
                inst.sync_info = si
                for ev in reversed(evs):
                    blk.instructions.insert(idx, ev)


def _build_kernel(ctx, tc, src, trg, wqt, wkt, wvt, out):
    nc = tc.nc
    ep = ctx.enter_context

    pool_const = ep(tc.tile_pool(name="const", bufs=1))
    pool_wtmp = ep(tc.tile_pool(name="wtmp", bufs=1))
    pool_sf = ep(tc.tile_pool(name="sf", bufs=1))
    pool_tf = ep(tc.tile_pool(name="tf", bufs=1))
    pool_sbf = ep(tc.tile_pool(name="sbf", bufs=2))
    pool_tbf = ep(tc.tile_pool(name="tbf", bufs=2))
    pool_qk = ep(tc.tile_pool(name="qk", bufs=1))
    pool_v = ep(tc.tile_pool(name="vpool", bufs=1))
    # 32 bufs: both s-halves' p tiles must be live simultaneously, since
    # half 1's scores are emitted before half 0's AV (a smaller ring makes
    # half-1 exp wait on half-0 AV matmuls that sit *behind* half-1's Z
    # matmuls in the in-order PE queue -> deadlock).
    pool_p = ep(tc.tile_pool(name="ppool", bufs=32))
    pool_zi = ep(tc.tile_pool(name="zipool", bufs=2))
    pool_stat = ep(tc.tile_pool(name="stat", bufs=2))
    pool_stat2 = ep(tc.tile_pool(name="stat2", bufs=2))
    pool_junk = ep(tc.tile_pool(name="junk", bufs=2))
    pool_out = ep(tc.tile_pool(name="outio", bufs=2))
    ps_s = ep(tc.tile_pool(name="ps_s", bufs=2, space="PSUM"))
    ps_z = ep(tc.tile_pool(name="ps_z", bufs=2, space="PSUM"))

    # ---- constants / weights (once) ----
    ones_bf = pool_const.tile([P, P], DT.bfloat16, name="ones_bf")
    nc.vector.memset(ones_bf[:], 1.0)
    dummy_bf = pool_const.tile([P, 512], DT.bfloat16, name="dummy_bf")
    nc.vector.memset(dummy_bf[:], 0.0)
    ones_f32 = pool_const.tile([P, P], DT.float32, name="ones_f32")
    nc.vector.memset(ones_f32[:], 1.0)

    def warm_mm(rhs_ap=None):
        """One dummy matmul to keep the PE HAM clock-gate warm.  If rhs_ap
        is given the matmul waits for it, pacing the dummy to just after
        that producer; otherwise it runs as soon as the PE is free."""
        rhs = rhs_ap if rhs_ap is not None else dummy_bf[:]
        free = rhs.shape[-1]
        lhsT = ones_f32[:] if rhs.dtype == DT.float32 else ones_bf[:]
        dps = ps_s.tile([P, 512], DT.float32, name="sps", tag="sps")
        nc.tensor.matmul(
            dps[:, 0:free],
            lhsT=lhsT,
            rhs=rhs,
            start=True,
            stop=True,
            skip_group_check=True,
        )

    # weight layout in SBUF: [128 (c within chunk), NCH*C (cchunk-major, d)]
    wq_bf = pool_const.tile([P, NCH * C], DT.bfloat16, name="wq_bf")
    wk_bf = pool_const.tile([P, NCH * C], DT.bfloat16, name="wk_bf")
    wv_bf = pool_const.tile([P, NCH * C], DT.bfloat16, name="wv_bf")
    for w_bf, w_d, wnm in ((wq_bf, wqt, "q"), (wk_bf, wkt, "k"), (wv_bf, wvt, "v")):
        wtmp = pool_wtmp.tile([P, NCH * C], DT.float32, name=f"wtmp_{wnm}")
        nc.gpsimd.dma_start(
            wtmp[:].rearrange("p (a d) -> p a d", a=NCH),
            w_d[:].rearrange("(a p) d -> p a d", p=P),
        )
        nc.vector.tensor_copy(w_bf[:], wtmp[:])

    # ~4.3us of back-to-back dummy matmuls at kernel start: push the HAM
    # through its cold window while the first DMAs are in flight.
    for _ in range(10):
        warm_mm()

    st = [dict() for _ in range(B_SH)]

    def prologue_io(b, fast):
        """DMA loads + f32->bf16 casts.  fast=True (batch 0, nothing else
        running) splits casts across DVE and ScalarE; otherwise they go to
        the idle GpSimd engine so DVE/ScalarE stay free for compute."""
        s = st[b]
        s["s_f"], s["t_f"], s["s_bf"], s["t_bf"] = [], [], [], []
        for cc in range(NCH):
            sf = pool_sf.tile([P, T], DT.float32, name=f"s_f{cc}")
            nc.gpsimd.dma_start(sf[:], src[b, cc * P : (cc + 1) * P, :])
            s["s_f"].append(sf)
            tf = pool_tf.tile([P, T], DT.float32, name=f"t_f{cc}")
            nc.sync.dma_start(tf[:], trg[b, cc * P : (cc + 1) * P, :])
            s["t_f"].append(tf)
        if fast:
            # pace a dummy matmul off the first DMA so the PE HAM gate
            # doesn't see a >3.4us idle window during the load phase.
            # batch 0's casts are emitted inside prologue_stats, interleaved
            # with the bn_stats, so stats start as soon as each DMA lands.
            warm_mm(s["s_f"][0][:, 0:128])
        else:
            for cc in range(NCH):
                sb = pool_sbf.tile([P, T], DT.bfloat16, name=f"s_bf{cc}")
                tb = pool_tbf.tile([P, T], DT.bfloat16, name=f"t_bf{cc}")
                for hh in range(2):
                    sl = slice(1024 * hh, 1024 * (hh + 1))
                    nc.gpsimd.tensor_copy(sb[:, sl], s["s_f"][cc][:, sl])
                    nc.gpsimd.tensor_copy(tb[:, sl], s["t_f"][cc][:, sl])
                s["s_bf"].append(sb)
                s["t_bf"].append(tb)

    def rowstats(b, x_bf, nm):
        """-> (mean [P,1] f32 AP, inv_sd [P,1] f32 tile) per row over T."""
        bnst = pool_stat.tile([P, 4 * 6], DT.float32, name=f"bnst_{nm}")
        for j in range(4):
            nc.vector.bn_stats(bnst[:, 6 * j : 6 * (j + 1)], x_bf[:, 512 * j : 512 * (j + 1)])
        mv = pool_stat.tile([P, 2], DT.float32, name=f"mv_{nm}")
        nc.vector.bn_aggr(mv[:], bnst[:])
        if b == 0:
            warm_mm(bnst[:, 0:24])  # HAM pacing tick through the stats phase
        sd = pool_stat.tile([P, 1], DT.float32, name=f"sd_{nm}")
        # sd = sqrt(var_pop * T/(T-1)) + EPS
        nc.scalar.activation(sd[:], mv[:, 1:2], ACTF.Sqrt, scale=float(T) / (T - 1))
        sde = pool_stat.tile([P, 1], DT.float32, name=f"sde_{nm}")
        nc.vector.tensor_scalar_add(sde[:], sd[:], EPS)
        inv = pool_stat.tile([P, 1], DT.float32, name=f"inv_{nm}")
        nc.vector.reciprocal(inv[:], sde[:])
        return mv[:, 0:1], inv

    def prologue_stats(b):
        s = st[b]
        fast = b == 0

        def cast_fast(src_f, nm):
            xb = pool_sbf.tile([P, T], DT.bfloat16, name=nm) if nm.startswith("s") \
                else pool_tbf.tile([P, T], DT.bfloat16, name=nm)
            for hh in range(2):
                sl = slice(1024 * hh, 1024 * (hh + 1))
                nc.vector.tensor_copy(xb[:, sl], src_f[:, sl])
                warm_mm(xb[:, 1024 * hh : 1024 * hh + 512])
            return xb

        s["mean_s"], s["inv_s"], s["mean_t"], s["inv_t"] = [], [], [], []
        if fast:
            s["s_bf"], s["t_bf"] = [], []
        for cc in range(NCH):
            if fast:
                s["s_bf"].append(cast_fast(s["s_f"][cc], f"s_bf{cc}"))
            m, i = rowstats(b, s["s_bf"][cc], f"s{cc}")
            s["mean_s"].append(m)
            s["inv_s"].append(i)
        for cc in range(NCH):
            if fast:
                s["t_bf"].append(cast_fast(s["t_f"][cc], f"t_bf{cc}"))
            m, i = rowstats(b, s["t_bf"][cc], f"t{cc}")
            s["mean_t"].append(m)
            s["inv_t"].append(i)

        # wq_s[c, d] = wqt[c, d] * inv_s[c]  (bf16), same for wk_s with inv_t
        # weight scaling on ScalarE (per-partition scale AP): DVE
        # tensor_scalar with an AP scalar measured 2.4us per chunk and sat on
        # the critical path to the first projection.
        wq_s = pool_stat.tile([P, NCH * C], DT.bfloat16, name="wq_s")
        wk_s = pool_stat.tile([P, NCH * C], DT.bfloat16, name="wk_s")
        mi_s, mi_t = [], []
        for cc in range(NCH):
            nc.scalar.activation(
                wq_s[:, cc * C : (cc + 1) * C], wq_bf[:, cc * C : (cc + 1) * C],
                ACTF.Identity, scale=s["inv_s"][cc][:],
            )
            nc.scalar.activation(
                wk_s[:, cc * C : (cc + 1) * C], wk_bf[:, cc * C : (cc + 1) * C],
                ACTF.Identity, scale=s["inv_t"][cc][:],
            )
            mis = pool_stat2.tile([P, 1], DT.bfloat16, name=f"mi_s{cc}")
            nc.vector.tensor_tensor(mis[:], s["mean_s"][cc], s["inv_s"][cc][:], ALU.mult)
            mi_s.append(mis)
            mit = pool_stat2.tile([P, 1], DT.bfloat16, name=f"mi_t{cc}")
            nc.vector.tensor_tensor(mit[:], s["mean_t"][cc], s["inv_t"][cc][:], ALU.mult)
            mi_t.append(mit)
        s["wq_s"], s["wk_s"], s["mi_s"], s["mi_t"] = wq_s, wk_s, mi_s, mi_t

        # PE pre-touches: pull cross-engine operand-ready waits off the first
        # real matmuls (MM encoding allows at most 2 sync waits).  Batch 0
        # only: for batch 1 the scheduler hoists these into the middle of
        # batch 0's score stream where they head-of-line block the PE on the
        # slow gpsimd input casts (measured 14us stall).
        if b == 0:
            for ap in (s["s_bf"][0], s["s_bf"][1], s["t_bf"][0], s["t_bf"][1]):
                nc.tensor.ldweights(weights=ap[:, 0:P])
            for ap in (wq_s, wk_s):
                nc.tensor.ldweights(weights=ap[:, 0:P])
            for ap in (mi_s[0], mi_s[1], mi_t[0], mi_t[1]):
                nc.tensor.ldweights(weights=ap[:])

        # beta[d] = sum_c w_s[c,d] * (mu[c]*inv[c]); psum [P, NCH] (d-chunk cols)
        negb = []
        for w_s, mi, nm in ((wq_s, mi_s, "q"), (wk_s, mi_t, "k")):
            bps = ps_s.tile([P, NCH], DT.float32, name="sps", tag="sps")
            for dc in range(NCH):
                for cc in range(NCH):
                    nc.tensor.matmul(
                        bps[:, dc : dc + 1],
                        lhsT=w_s[:, cc * C + dc * P : cc * C + (dc + 1) * P],
                        rhs=mi[cc][:],
                        start=(cc == 0),
                        stop=(cc == NCH - 1),
                    )
            nb = pool_stat2.tile([P, NCH], DT.float32, name=f"negb_{nm}")
            nc.vector.tensor_scalar_mul(nb[:], bps[:], -1.0)
            negb.append(nb)
        s["negbq"], s["negbk"] = negb

    def proj_qk(b):
        # Qt/Kt: [d, t] bf16 (per d-chunk tiles), bias folded during eviction
        s = st[b]
        s["qt_bf"], s["kt_bf"] = [], []
        for w_s, nb, outk, nm in (
            (s["wq_s"], s["negbq"], "qt_bf", "qt"),
            (s["wk_s"], s["negbk"], "kt_bf", "kt"),
        ):
            x_bf = s["s_bf"] if nm == "qt" else s["t_bf"]
            for dc in range(NCH):
                ot = pool_qk.tile([P, T], DT.bfloat16, name=f"{nm}{dc}")
                for half in range(2):
                    pps = ps_s.tile([P, 1024], DT.float32, name="sps", tag="sps")
                    for cc in range(NCH):
                        for n4 in range(2):
                            nc.tensor.matmul(
                                pps[:, 512 * n4 : 512 * (n4 + 1)],
                                lhsT=w_s[:, cc * C + dc * P : cc * C + (dc + 1) * P],
                                rhs=x_bf[cc][:, 1024 * half + 512 * n4 : 1024 * half + 512 * (n4 + 1)],
                                start=(cc == 0),
                                stop=(cc == NCH - 1),
                            )
                    nc.scalar.activation(
                        ot[:, 1024 * half : 1024 * (half + 1)],
                        pps[:],
                        ACTF.Identity,
                        bias=nb[:, dc : dc + 1],
                        scale=1.0,
                    )
                s[outk].append(ot)

    def proj_v(b):
        # V_T: [t within chunk, tchunk-major d]  (v_bf[p, 256*j + d])
        s = st[b]
        v_bf = pool_v.tile([P, NTCH * C], DT.bfloat16, name="v_bf")
        v2_bf = pool_v.tile([P, NTCH * C], DT.bfloat16, name="v2_bf")
        for g in range(4):
            vps = ps_s.tile([P, 1024], DT.float32, name="sps", tag="sps")
            for j4 in range(4):
                j = 4 * g + j4
                for cc in range(NCH):
                    nc.tensor.matmul(
                        vps[:, 256 * j4 : 256 * (j4 + 1)],
                        lhsT=s["t_bf"][cc][:, P * j : P * (j + 1)],
                        rhs=wv_bf[:, cc * C : (cc + 1) * C],
                        start=(cc == 0),
                        stop=(cc == NCH - 1),
                    )
            nc.vector.tensor_copy(v_bf[:, 1024 * g : 1024 * (g + 1)], vps[:])
        # v^2 on GpSimd: a same-tensor tensor_tensor measured 7.9us on DVE
        # (port conflict); GpSimd is idle here and off the critical path.
        nc.gpsimd.tensor_tensor(v2_bf[:], v_bf[:], v_bf[:], ALU.mult)
        s["v_bf"], s["v2_bf"] = v_bf, v2_bf

    def stats_init(b):
        s = st[b]
        s["sm_h"] = pool_stat.tile([P, 2 * NCH], DT.float32, name="sm_h")
        s["sm2_h"] = pool_stat.tile([P, 2 * NCH], DT.float32, name="sm2_h")
        s["a_uh"] = pool_stat.tile([P, 2 * NTCH], DT.float32, name="a_uh")
        s["p_t"] = [[], []]
        s["z_ps"] = [None, None]

    def scores_half(b, sh):
        """scores^T -> exp -> Z accumulation for s-half sh."""
        s = st[b]
        so = 1024 * sh
        z_ps = ps_z.tile([P, 1024], DT.float32, name="zav", tag="zav")
        s["z_ps"][sh] = z_ps
        for tch in range(NTCH):
            p = pool_p.tile([P, 1024], DT.bfloat16, name="p")
            sps = ps_s.tile([P, 1024], DT.float32, name="sps", tag="sps")
            for dc in range(NCH):
                for n2 in range(2):
                    nc.tensor.matmul(
                        sps[:, 512 * n2 : 512 * (n2 + 1)],
                        lhsT=s["kt_bf"][dc][:, P * tch : P * (tch + 1)],
                        rhs=s["qt_bf"][dc][:, so + 512 * n2 : so + 512 * (n2 + 1)],
                        start=(dc == 0),
                        stop=(dc == NCH - 1),
                    )
            nc.scalar.activation(p[:], sps[:], ACTF.Exp, scale=1.0 / 16.0)
            for n2 in range(2):
                nc.tensor.matmul(
                    z_ps[:, 512 * n2 : 512 * (n2 + 1)],
                    lhsT=ones_bf[:],
                    rhs=p[:, 512 * n2 : 512 * (n2 + 1)],
                    start=(tch == 0),
                    stop=(tch == NTCH - 1),
                    skip_group_check=True,
                )
            s["p_t"][sh].append(p)

    def av_half(b, sh):
        """normalize p by 1/Z (accumulating a_u) and contract attn @ v."""
        s = st[b]
        z_ps = s["z_ps"][sh]
        # 1/Z as exp(-ln Z) on ScalarE: DVE reciprocal measured 6.7us per
        # [128,1024] tile and gated every AV phase; two ACT passes cost ~2us
        # on the less-loaded scalar queue.  (reciprocal_approx_fast, the
        # custom DVE op, fails neuronxcc codegen on this toolchain.)
        zln = pool_zi.tile([P, 1024], DT.float32, name="zln")
        nc.scalar.activation(zln[:], z_ps[:], ACTF.Ln)
        zinv = pool_zi.tile([P, 1024], DT.float32, name="zinv")
        nc.scalar.activation(zinv[:], zln[:], ACTF.Exp, scale=-1.0)

        for i, p in enumerate(s["p_t"][sh]):
            # out = (P * 1) * zinv (normalize in place); accum -> a_u half
            nc.vector.scalar_tensor_tensor(
                out=p[:],
                in0=p[:],
                scalar=1.0,
                in1=zinv[:],
                op0=ALU.mult,
                op1=ALU.mult,
                accum_out=s["a_uh"][:, NTCH * sh + i : NTCH * sh + i + 1],
            )

        for dc in range(NCH):
            avps = ps_s.tile([P, 1024], DT.float32, name="sps", tag="sps")
            for tch in range(NTCH):
                for n2 in range(2):
                    nc.tensor.matmul(
                        avps[:, 512 * n2 : 512 * (n2 + 1)],
                        lhsT=s["v_bf"][:, 256 * tch + P * dc : 256 * tch + P * (dc + 1)],
                        rhs=s["p_t"][sh][tch][:, 512 * n2 : 512 * (n2 + 1)],
                        start=(tch == 0),
                        stop=(tch == NTCH - 1),
                    )
            junk = pool_junk.tile([P, 1024], DT.bfloat16, name="junk")
            nc.scalar.activation(
                junk[:], avps[:], ACTF.Square,
                accum_out=s["sm2_h"][:, NCH * sh + dc : NCH * sh + dc + 1],
            )
            nc.vector.reduce_sum(
                s["sm_h"][:, NCH * sh + dc : NCH * sh + dc + 1], avps[:],
                axis=mybir.AxisListType.X,
            )

    def finals(b):
        s = st[b]
        # combine halves
        a_u = pool_stat.tile([P, NTCH], DT.float32, name="a_u")
        nc.vector.tensor_add(a_u[:], s["a_uh"][:, 0:NTCH], s["a_uh"][:, NTCH : 2 * NTCH])
        a_ub = pool_stat2.tile([P, NTCH], DT.bfloat16, name="a_ub")
        nc.vector.tensor_copy(a_ub[:], a_u[:])
        sm = pool_stat.tile([P, NCH], DT.float32, name="sm")
        nc.vector.tensor_add(sm[:], s["sm_h"][:, 0:NCH], s["sm_h"][:, NCH : 2 * NCH])
        sm2 = pool_stat.tile([P, NCH], DT.float32, name="sm2")
        nc.vector.tensor_add(sm2[:], s["sm2_h"][:, 0:NCH], s["sm2_h"][:, NCH : 2 * NCH])

        # attn@v^2 matvec (per-partition result)
        av2_ps = ps_s.tile([P, NCH], DT.float32, name="sps", tag="sps")
        for dc in range(NCH):
            for j in range(NTCH):
                nc.tensor.matmul(
                    av2_ps[:, dc : dc + 1],
                    lhsT=s["v2_bf"][:, 256 * j + P * dc : 256 * j + P * (dc + 1)],
                    rhs=a_ub[:, j : j + 1],
                    start=(j == 0),
                    stop=(j == NTCH - 1),
                )

        # ================= finals + output ================================
        for dc in range(NCH):
            d1 = pool_stat.tile([P, 1], DT.float32, name=f"d1_{dc}")
            nc.vector.tensor_tensor(d1[:], av2_ps[:, dc : dc + 1], sm2[:, dc : dc + 1], ALU.subtract)
            r1 = pool_stat.tile([P, 1], DT.float32, name=f"r1_{dc}")
            nc.vector.tensor_scalar_max(r1[:], d1[:], 0.0)
            stdv = pool_stat.tile([P, 1], DT.float32, name=f"std_{dc}")
            nc.scalar.activation(stdv[:], r1[:], ACTF.Sqrt, scale=1.0 / T)
            av = pool_stat.tile([P, 1], DT.float32, name=f"av_{dc}")
            nc.vector.tensor_tensor(av[:], stdv[:], s["inv_s"][dc][:], ALU.mult)
            musc = pool_stat.tile([P, 1], DT.float32, name=f"musc_{dc}")
            nc.vector.tensor_scalar_mul(musc[:], sm[:, dc : dc + 1], 1.0 / T)
            negms = pool_stat.tile([P, 1], DT.float32, name=f"negms_{dc}")
            nc.vector.tensor_scalar_mul(negms[:], s["mean_s"][dc], -1.0)
            bv = pool_stat.tile([P, 1], DT.float32, name=f"bv_{dc}")
            nc.vector.scalar_tensor_tensor(
                out=bv[:], in0=av[:], scalar=negms[:], in1=musc[:], op0=ALU.mult, op1=ALU.add
            )
            for half in range(2):
                # out = av*src + bv on DVE (fused dual-scalar op); the scalar
                # queue is still draining the last exps at this point.
                o_sb = pool_out.tile([P, 1024], DT.float32, name="o_sb")
                nc.vector.tensor_scalar(
                    o_sb[:],
                    s["s_bf"][dc][:, 1024 * half : 1024 * (half + 1)],
                    av[:],
                    bv[:],
                    ALU.mult,
                    ALU.add,
                )
                nc.sync.dma_start(
                    out[b, dc * P : (dc + 1) * P, 1024 * half : 1024 * (half + 1)], o_sb[:]
                )

    # ================= software-pipelined emission ======================
    prologue_io(0, fast=True)
    prologue_stats(0)
    stats_init(0)
    proj_v(0)  # V needs no stats: fills the PE while the QK path waits
    proj_qk(0)
    scores_half(0, 0)
    prologue_io(1, fast=False)
    scores_half(0, 1)
    av_half(0, 0)
    prologue_stats(1)
    stats_init(1)
    proj_qk(1)
    av_half(0, 1)
    finals(0)
    # proj_v(1) must come after finals(0): v2_bf(1)'s DVE write reuses
    # v2_bf(0)'s slot, whose last reader is the av2 matvec in finals(0) --
    # and that matvec needs a_ub from DVE instructions that would otherwise
    # queue behind the v2 multiply.
    proj_v(1)
    scores_half(1, 0)
    scores_half(1, 1)
    av_half(1, 0)
    av_half(1, 1)
    finals(1)


_NC_CACHE = None


def _get_nc():
    global _NC_CACHE
    if _NC_CACHE is None:
        _NC_CACHE = _build_nc()
    return _NC_CACHE


def _run(src, trg, Wq, Wk, Wv, **kwargs):
    src = np.ascontiguousarray(np.asarray(src, dtype=np.float32))
    trg = np.ascontiguousarray(np.asarray(trg, dtype=np.float32))
    wqt = np.ascontiguousarray(np.asarray(Wq, dtype=np.float32).T)
    wkt = np.ascontiguousarray(np.asarray(Wk, dtype=np.float32).T)
    wvt = np.ascontiguousarray(np.asarray(Wv, dtype=np.float32).T)
    nc = _get_nc()
    in_maps = [
        {
            "src": src[i * B_SH : (i + 1) * B_SH],
            "trg": trg[i * B_SH : (i + 1) * B_SH],
            "wqt": wqt,
            "wkt": wkt,
            "wvt": wvt,
        }
        for i in range(N_CORES)
    ]
    res = run_bass_kernel_spmd(nc, in_maps, list(range(N_CORES)), **kwargs)
    outp = np.concatenate([res.results[i]["out"] for i in range(N_CORES)], axis=0)
    return outp.astype(np.float32), res


def kernel(src, trg, Wq, Wk, Wv):
    outp, _ = _run(src, trg, Wq, Wk, Wv)
    return outp
